# revision 12
# baseline (speedup 1.0000x reference)
"""Multi-head attention (B=2, S=2048, D=768, H=12) on 8 Trainium2 NeuronCores.

Sharding: core c handles batch b=c//4 and heads 3*(c%4) .. 3*(c%4)+2.
Each core:
  1. Projects Q,K (feature-major, transposed) and V (sequence-major, with an
     appended ones-column for the softmax denominator) for its 3 heads.
  2. Computes scores^T = K @ Q^T per head (contraction over head_dim=64, heads
     paired into PE row-groups), exp on ScalarE (scores are O(1), no max
     subtraction needed), then ctx^T_aug = V_aug^T @ exp(scores^T) which yields
     both the unnormalized context and the softmax denominator in one pass.
  3. Normalizes, writes local ctx^T [192, 2048] to DRAM.
  4. One 8-rank AllGather -> ctx^T for all heads/batches [1536, 2048].
  5. Indirect-gathers its (batch, s_q quarter) slice and computes the output
     projection y^T[:, q*512:(q+1)*512] = Wo^T @ ctx^T + bo.
Host assembles y[b, q*512:(q+1)*512, :] = out_c^T.

All matmul operands are float32r (TF32-like, full PE rate); accumulation fp32.
"""
import sys

if "/opt/trn_rl_repo" not in sys.path:
    sys.path.insert(0, "/opt/trn_rl_repo")

import numpy as np

B, S, D, H = 2, 2048, 768, 12
HD = 64
P = 128
N_CORES = 8
HPC = 3          # heads per core
NQ = 4           # s_q chunks of 512
SK = 16          # s_k chunks of 128
KD = 6           # D chunks of 128
W = 512          # working free-dim chunk

_CACHE = {}


def _install_profile_shim():
    """run_bass_kernel_spmd(trace=True) needs antenv.axon_hooks; provide it."""
    import contextlib
    import ctypes
    import types

    if "antenv.axon_hooks" in sys.modules:
        return
    try:
        lib = ctypes.CDLL("/opt/axon/libaxon_pjrt.so")
    except OSError:
        return
    if not hasattr(lib, "axon_start_nrt_profile"):
        return
    lib.axon_start_nrt_profile.argtypes = [
        ctypes.POINTER(ctypes.c_int64),
        ctypes.c_size_t,
    ]
    lib.axon_start_nrt_profile.restype = ctypes.c_int64
    lib.axon_stop_nrt_profile.argtypes = [ctypes.c_char_p]
    lib.axon_stop_nrt_profile.restype = ctypes.c_int64

    @contextlib.contextmanager
    def _hook(output_dir, device_ids):
        import jax

        jax.devices()
        if device_ids:
            ids = (ctypes.c_int64 * len(device_ids))(*device_ids)
            rc = lib.axon_start_nrt_profile(ids, len(device_ids))
        else:
            rc = lib.axon_start_nrt_profile(None, 0)
        if rc != 0:
            raise RuntimeError(f"axon_start_nrt_profile rc={rc}")
        try:
            yield
        finally:
            n = lib.axon_stop_nrt_profile(str(output_dir).encode())
            if n < 0:
                raise RuntimeError(f"axon_stop_nrt_profile rc={n}")

    mod = types.ModuleType("antenv.axon_hooks")
    mod.get_axon_ntff_profile_hook = lambda: _hook
    mod.set_axon_ntff_profile_hook = lambda h: None
    sys.modules["antenv.axon_hooks"] = mod


def _build():
    import concourse.bass as bass
    from concourse import bacc
    import concourse.tile as tile
    import concourse.mybir as mybir

    f32r = mybir.dt.float32r
    f32 = mybir.dt.float32
    u32 = mybir.dt.uint32
    AF = mybir.ActivationFunctionType
    ALU = mybir.AluOpType

    nc = bacc.Bacc("TRN2", target_bir_lowering=False, debug=False,
                   num_devices=N_CORES)

    xT = nc.dram_tensor("xT", [D, S], f32r, kind="ExternalInput")
    w_qk = nc.dram_tensor("w_qk", [D, 384], f32r, kind="ExternalInput")
    b_qk = nc.dram_tensor("b_qk", [384, 1], f32, kind="ExternalInput")
    w_v = nc.dram_tensor("w_v", [D, 256], f32r, kind="ExternalInput")
    b_v = nc.dram_tensor("b_v", [1, 256], f32, kind="ExternalInput")
    w_o = nc.dram_tensor("w_o", [D, D], f32r, kind="ExternalInput")
    b_o = nc.dram_tensor("b_o", [D, 1], f32, kind="ExternalInput")
    gidx = nc.dram_tensor("gidx", [D, 1], u32, kind="ExternalInput")
    zin = nc.dram_tensor("zin", [P, P], f32r, kind="ExternalInput")
    out = nc.dram_tensor("out", [D, W], f32r, kind="ExternalOutput")

    cc_in = nc.dram_tensor("cc_in", [NQ, HPC * HD, W], f32r)
    cc_all = nc.dram_tensor("cc_all", [NQ * N_CORES * HPC * HD, W], f32r,
                            addr_space="Shared")

    with tile.TileContext(nc) as tc:
        with tc.tile_pool(name="const", bufs=1) as const, \
             tc.tile_pool(name="qkp", bufs=1) as qkp, \
             tc.tile_pool(name="vp", bufs=1) as vp, \
             tc.tile_pool(name="work", bufs=4) as work, \
             tc.tile_pool(name="expp", bufs=4) as expp, \
             tc.tile_pool(name="gat", bufs=1) as gat, \
             tc.tile_pool(name="outp", bufs=3) as outp:

            # ---- constant loads -------------------------------------------
            zeros_t = const.tile([P, P], f32r, tag="zeros")
            nc.sync.dma_start(out=zeros_t, in_=zin[:, :])
            wqk = []
            xt = []
            for k in range(KD):
                t = const.tile([P, 384], f32r, tag=f"wqk{k}")
                nc.sync.dma_start(out=t, in_=w_qk[k * P:(k + 1) * P, :])
                wqk.append(t)
            for k in range(KD):
                t = const.tile([P, S], f32r, tag=f"xt{k}", name=f"xt{k}")
                xt.append(t)
            for n in range(NQ):
                for k in range(KD):
                    nc.sync.dma_start(
                        out=xt[k][:, n * W:(n + 1) * W],
                        in_=xT[k * P:(k + 1) * P, n * W:(n + 1) * W])
            wv = []
            for k in range(KD):
                t = const.tile([P, 256], f32r, tag=f"wv{k}")
                nc.sync.dma_start(out=t, in_=w_v[k * P:(k + 1) * P, :])
                wv.append(t)
            bqk = []
            for m in range(3):
                t = const.tile([P, 1], f32, tag=f"bqk{m}")
                nc.sync.dma_start(out=t, in_=b_qk[m * P:(m + 1) * P, :])
                bqk.append(t)
            bv = const.tile([P, 256], f32, tag="bv")
            bv_bcast = bass.AP(tensor=b_v[:, :].tensor, offset=0,
                               ap=[[0, P], [1, 256]])
            nc.gpsimd.dma_start(out=bv, in_=bv_bcast)
            wo = []
            bo = []
            gix = []
            for k in range(KD):
                t = const.tile([P, D], f32r, tag=f"wo{k}")
                nc.sync.dma_start(out=t, in_=w_o[k * P:(k + 1) * P, :])
                wo.append(t)
                t = const.tile([P, 1], f32, tag=f"bo{k}")
                nc.sync.dma_start(out=t, in_=b_o[k * P:(k + 1) * P, :])
                bo.append(t)
                t = const.tile([P, 1], u32, tag=f"gix{k}")
                nc.sync.dma_start(out=t, in_=gidx[k * P:(k + 1) * P, :])
                gix.append(t)

            # ---- attention -----------------------------------------------
            # Chunk = one [s_k 128, s_q 512] score block for one head.
            # Groups of 2 chunks share a 2-bank PSUM tile so one ACT exp
            # covers 1024 columns (amortizes the ~352-cycle ACT overhead).
            # Software-pipelined emission: mm_s(g+1) is emitted before
            # mm_c(g) so the PE never stalls behind the ACT.
            qkt = [qkp.tile([P, S], f32r, tag=f"qkt{m}", name=f"qkt{m}")
                   for m in range(3)]
            q2c = qkp.tile([64, S], f32r, tag="q2c")
            vsb = [vp.tile([P, 256], f32r, tag=f"v{s}", name=f"v{s}")
                   for s in range(SK)]

            def normalize(pc, nq, h):
                rec = work.tile([1, W], f32, tag="rec")
                nc.vector.reciprocal(rec[0:1, :], pc[64:65, :])
                rb = work.tile([64, W], f32, tag="rb")
                nc.gpsimd.partition_broadcast(rb, rec[:1, :])
                ctx = work.tile([64, W], f32r, tag="ctx")
                nc.vector.tensor_tensor(out=ctx, in0=pc[0:64, :], in1=rb,
                                        op=ALU.mult)
                nc.sync.dma_start(
                    out=cc_in[nq, h * HD:(h + 1) * HD, :],
                    in_=ctx)

            # build group list: per nq, pair phase then solo phase
            groups = []
            for nq in range(NQ):
                for sk in range(SK):
                    groups.append({"nq": nq, "chunks": [(0, sk), (1, sk)],
                                   "last": False})
                for sk in range(0, SK, 2):
                    g = {"nq": nq, "chunks": [(2, sk), (2, sk + 1)],
                         "last": sk == SK - 2}
                    groups.append(g)

            pc_tiles = {}
            cnt = {}
            norm_done = {}
            ag_fired = set()

            def emit_mm_s(gi, grp):
                nq = grp["nq"]
                eps = psE.tile([P, 2 * W], f32, tag="ea" if gi % 2 == 0
                               else "eb", name=f"eps{gi}")
                for j, (h, sk) in enumerate(grp["chunks"]):
                    if h == 0:
                        lhsT = qkt[0][0:64, sk * P:(sk + 1) * P]
                        rhs = qkt[1][0:64, nq * W:(nq + 1) * W]
                    elif h == 1:
                        lhsT = qkt[0][64:128, sk * P:(sk + 1) * P]
                        rhs = qkt[1][64:128, nq * W:(nq + 1) * W]
                    else:
                        lhsT = qkt[2][0:64, sk * P:(sk + 1) * P]
                        rhs = q2c[:, nq * W:(nq + 1) * W]
                    nc.tensor.matmul(eps[:, j * W:(j + 1) * W], lhsT, rhs,
                                     start=True, stop=True)
                esb = expp.tile([P, 2 * W], f32r, tag="e", name=f"esb{gi}")
                nc.scalar.activation(esb, eps, AF.Exp)
                return esb

            def emit_mm_c(grp, esb):
                nq = grp["nq"]
                for j, (h, sk) in enumerate(grp["chunks"]):
                    key = (nq, h)
                    if key not in pc_tiles:
                        pc_tiles[key] = psC.tile([65, W], f32, tag="pc",
                                                 name=f"pc{nq}_{h}")
                        cnt[key] = 0
                    nc.tensor.matmul(
                        pc_tiles[key],
                        vsb[sk][:, h * 65:h * 65 + 65],
                        esb[:, j * W:(j + 1) * W],
                        start=(cnt[key] == 0), stop=(cnt[key] == SK - 1))
                    cnt[key] += 1
                    if cnt[key] == SK:
                        normalize(pc_tiles[key], nq, h)
                        norm_done.setdefault(nq, set()).add(h)
                        if norm_done[nq] == {0, 1, 2}:
                            nc.gpsimd.collective_compute(
                                "AllGather",
                                ALU.bypass,
                                ins=[cc_in[nq]],
                                outs=[cc_all[nq * 1536:(nq + 1) * 1536, :]],
                                replica_groups=[list(range(N_CORES))],
                            )

            with tc.tile_pool(name="ps_proj", bufs=4, space="PSUM") as psP:

                def emit_qk_block(n):
                    for m in range(3):
                        ps = psP.tile([P, W], f32, tag="proj",
                                      name=f"psqk{n}_{m}")
                        first = n == 0 and m == 0
                        if first:
                            # zero-contribution matmuls: keep the PE busy
                            # while x/w DMAs land so HAM reaches full clock.
                            # They cover the whole [0:512] region with zeros
                            # so the real accumulation below lands on clean
                            # has_written state either way.
                            for d in range(24):
                                if d % 2 == 0:
                                    nc.tensor.matmul(
                                        ps[:, 0:384], zeros_t,
                                        wqk[0][:, :],
                                        start=(d == 0), stop=False,
                                        skip_group_check=True)
                                else:
                                    nc.tensor.matmul(
                                        ps[:, 384:512], zeros_t,
                                        wqk[1][:, 0:128],
                                        start=(d == 1), stop=False,
                                        skip_group_check=True)
                        for k in range(KD):
                            nc.tensor.matmul(
                                ps,
                                wqk[k][:, m * P:(m + 1) * P],
                                xt[k][:, n * W:(n + 1) * W],
                                start=(k == 0 and not first),
                                stop=(k == KD - 1),
                                skip_group_check=first)
                        nc.vector.tensor_scalar_add(
                            qkt[m][:, n * W:(n + 1) * W], ps, bqk[m])
                    nc.sync.dma_start(out=q2c[:, n * W:(n + 1) * W],
                                      in_=qkt[2][64:128, n * W:(n + 1) * W])

                def emit_v_block(n):
                    for s_ in range(4 * n, 4 * n + 4):
                        ps = psP.tile([P, W], f32, tag="proj",
                                      name=f"psv{s_}")
                        for k in range(KD):
                            nc.tensor.matmul(
                                ps[:, 0:256],
                                xt[k][:, s_ * P:(s_ + 1) * P],
                                wv[k],
                                start=(k == 0), stop=(k == KD - 1))
                        nc.vector.tensor_tensor(out=vsb[s_], in0=ps[:, 0:256],
                                                in1=bv, op=ALU.add)

                for n in range(NQ):
                    emit_qk_block(n)
                    emit_v_block(n)

            with tc.tile_pool(name="ps_e", bufs=1, space="PSUM") as psE, \
                 tc.tile_pool(name="ps_c", bufs=3, space="PSUM") as psC:
                prev = None
                for gi, grp in enumerate(groups):
                    esb = emit_mm_s(gi, grp)
                    if prev is not None:
                        emit_mm_c(prev[0], prev[1])
                    prev = (grp, esb)
                emit_mm_c(prev[0], prev[1])

            # ---- gather + output projection ------------------------------
            ctxg = []
            for k in range(KD):
                t = gat.tile([P, W], f32r, tag=f"ctxg{k}", name=f"ctxg{k}")
                nc.gpsimd.indirect_dma_start(
                    out=t,
                    out_offset=None,
                    in_=cc_all[:, :],
                    in_offset=bass.IndirectOffsetOnAxis(ap=gix[k][:, :1],
                                                        axis=0),
                )
                ctxg.append(t)
            with tc.tile_pool(name="ps_y", bufs=2, space="PSUM") as py:
                for m in range(KD):
                    ps = py.tile([P, W], f32)
                    if m == 0:
                        for d in range(16):
                            nc.tensor.matmul(
                                ps, zeros_t, wo[0][:, 0:W],
                                start=(d == 0), stop=False,
                                skip_group_check=True)
                    for k in range(KD):
                        nc.tensor.matmul(
                            ps,
                            wo[k][:, m * P:(m + 1) * P],
                            ctxg[k],
                            start=(k == 0 and m != 0),
                            stop=(k == KD - 1),
                            skip_group_check=(m == 0))
                    yt = outp.tile([P, W], f32r, tag="yt")
                    nc.vector.tensor_scalar_add(yt, ps, bo[m])
                    nc.sync.dma_start(out=out[m * P:(m + 1) * P, :], in_=yt)

    nc.compile()
    return nc


def _get_nc():
    if "nc" not in _CACHE:
        _install_profile_shim()
        _CACHE["nc"] = _build()
    return _CACHE["nc"]


def _make_in_maps(x, Wq, bq, Wk, bk, Wv, bv, Wo, bo):
    scale = np.float32(1.0 / np.sqrt(HD))
    f = np.float32
    x, Wq, bq, Wk, bk, Wv, bv, Wo, bo = [
        np.asarray(a, dtype=f) for a in (x, Wq, bq, Wk, bk, Wv, bv, Wo, bo)]

    in_maps = []
    for c in range(N_CORES):
        b = c // 4
        hs = (c % 4) * HPC
        q = c % 4
        hh = [hs, hs + 1, hs + 2]

        def wc(Wm, h):
            return Wm[:, h * HD:(h + 1) * HD]

        def bc(bm, h):
            return bm[h * HD:(h + 1) * HD]

        xTb = np.ascontiguousarray(x[b].T)
        w_qk = np.concatenate(
            [wc(Wk, hh[0]), wc(Wk, hh[1]),
             wc(Wq, hh[0]) * scale, wc(Wq, hh[1]) * scale,
             wc(Wk, hh[2]), wc(Wq, hh[2]) * scale], axis=1)
        b_qk = np.concatenate(
            [bc(bk, hh[0]), bc(bk, hh[1]),
             bc(bq, hh[0]) * scale, bc(bq, hh[1]) * scale,
             bc(bk, hh[2]), bc(bq, hh[2]) * scale])[:, None]
        w_v = np.zeros((D, 256), dtype=f)
        b_v = np.zeros((1, 256), dtype=f)
        for i, h in enumerate(hh):
            w_v[:, i * 65:i * 65 + HD] = wc(Wv, h)
            b_v[0, i * 65:i * 65 + HD] = bc(bv, h)
            b_v[0, i * 65 + HD] = 1.0
        i_feat = np.arange(D, dtype=np.uint32)
        g = q * 1536 + (4 * b + i_feat // 192) * 192 + (i_feat % 192)
        in_maps.append({
            "xT": np.ascontiguousarray(xTb),
            "w_qk": np.ascontiguousarray(w_qk),
            "b_qk": np.ascontiguousarray(b_qk),
            "w_v": w_v,
            "b_v": b_v,
            "w_o": np.ascontiguousarray(Wo),
            "b_o": np.ascontiguousarray(bo[:, None]),
            "gidx": g.astype(np.uint32)[:, None],
            "zin": np.zeros((P, P), dtype=f),
        })
    return in_maps


def kernel(x, Wq, bq, Wk, bk, Wv, bv, Wo, bo, _trace=False):
    from concourse.bass_utils import run_bass_kernel_spmd

    nc = _get_nc()
    in_maps = _make_in_maps(x, Wq, bq, Wk, bk, Wv, bv, Wo, bo)
    res = run_bass_kernel_spmd(nc, in_maps, list(range(N_CORES)),
                               trace=_trace)
    _CACHE["last_results"] = res
    y = np.empty((B, S, D), dtype=np.float32)
    for c in range(N_CORES):
        b = c // 4
        q = c % 4
        y[b, q * W:(q + 1) * W, :] = res.results[c]["out"].T
    return y


# revision 13
# speedup vs baseline: 1.0497x; 1.0497x over previous
"""Multi-head attention (B=2, S=2048, D=768, H=12) on 8 Trainium2 NeuronCores.

Sharding: core c handles batch b=c//4 and heads 3*(c%4) .. 3*(c%4)+2.
Each core:
  1. Projects Q,K (feature-major, transposed) and V (sequence-major, with an
     appended ones-column for the softmax denominator) for its 3 heads.
  2. Computes scores^T = K @ Q^T per head (contraction over head_dim=64, heads
     paired into PE row-groups), exp on ScalarE (scores are O(1), no max
     subtraction needed), then ctx^T_aug = V_aug^T @ exp(scores^T) which yields
     both the unnormalized context and the softmax denominator in one pass.
  3. Normalizes, writes local ctx^T [192, 2048] to DRAM.
  4. One 8-rank AllGather -> ctx^T for all heads/batches [1536, 2048].
  5. Indirect-gathers its (batch, s_q quarter) slice and computes the output
     projection y^T[:, q*512:(q+1)*512] = Wo^T @ ctx^T + bo.
Host assembles y[b, q*512:(q+1)*512, :] = out_c^T.

All matmul operands are float32r (TF32-like, full PE rate); accumulation fp32.
"""
import sys

if "/opt/trn_rl_repo" not in sys.path:
    sys.path.insert(0, "/opt/trn_rl_repo")

import numpy as np

B, S, D, H = 2, 2048, 768, 12
HD = 64
P = 128
N_CORES = 8
HPC = 3          # heads per core
NQ = 4           # s_q chunks of 512
SK = 16          # s_k chunks of 128
KD = 6           # D chunks of 128
W = 512          # working free-dim chunk

_CACHE = {}


def _install_profile_shim():
    """run_bass_kernel_spmd(trace=True) needs antenv.axon_hooks; provide it."""
    import contextlib
    import ctypes
    import types

    if "antenv.axon_hooks" in sys.modules:
        return
    try:
        lib = ctypes.CDLL("/opt/axon/libaxon_pjrt.so")
    except OSError:
        return
    if not hasattr(lib, "axon_start_nrt_profile"):
        return
    lib.axon_start_nrt_profile.argtypes = [
        ctypes.POINTER(ctypes.c_int64),
        ctypes.c_size_t,
    ]
    lib.axon_start_nrt_profile.restype = ctypes.c_int64
    lib.axon_stop_nrt_profile.argtypes = [ctypes.c_char_p]
    lib.axon_stop_nrt_profile.restype = ctypes.c_int64

    @contextlib.contextmanager
    def _hook(output_dir, device_ids):
        import jax

        jax.devices()
        if device_ids:
            ids = (ctypes.c_int64 * len(device_ids))(*device_ids)
            rc = lib.axon_start_nrt_profile(ids, len(device_ids))
        else:
            rc = lib.axon_start_nrt_profile(None, 0)
        if rc != 0:
            raise RuntimeError(f"axon_start_nrt_profile rc={rc}")
        try:
            yield
        finally:
            n = lib.axon_stop_nrt_profile(str(output_dir).encode())
            if n < 0:
                raise RuntimeError(f"axon_stop_nrt_profile rc={n}")

    mod = types.ModuleType("antenv.axon_hooks")
    mod.get_axon_ntff_profile_hook = lambda: _hook
    mod.set_axon_ntff_profile_hook = lambda h: None
    sys.modules["antenv.axon_hooks"] = mod


def _build():
    import concourse.bass as bass
    from concourse import bacc
    import concourse.tile as tile
    import concourse.mybir as mybir

    f32r = mybir.dt.float32r
    f32 = mybir.dt.float32
    u32 = mybir.dt.uint32
    AF = mybir.ActivationFunctionType
    ALU = mybir.AluOpType

    nc = bacc.Bacc("TRN2", target_bir_lowering=False, debug=False,
                   num_devices=N_CORES)

    xT = nc.dram_tensor("xT", [D, S], f32r, kind="ExternalInput")
    w_qk = nc.dram_tensor("w_qk", [D, 384], f32r, kind="ExternalInput")
    b_qk = nc.dram_tensor("b_qk", [384, 1], f32, kind="ExternalInput")
    w_v = nc.dram_tensor("w_v", [D, 256], f32r, kind="ExternalInput")
    b_v = nc.dram_tensor("b_v", [1, 256], f32, kind="ExternalInput")
    w_o = nc.dram_tensor("w_o", [D, D], f32r, kind="ExternalInput")
    b_o = nc.dram_tensor("b_o", [D, 1], f32, kind="ExternalInput")
    gidx = nc.dram_tensor("gidx", [D, 1], u32, kind="ExternalInput")
    zin = nc.dram_tensor("zin", [P, P], f32r, kind="ExternalInput")
    out = nc.dram_tensor("out", [D, W], f32r, kind="ExternalOutput")

    cc_in = nc.dram_tensor("cc_in", [NQ, HPC * HD, W], f32r)
    cc_all = nc.dram_tensor("cc_all", [NQ * N_CORES * HPC * HD, W], f32r,
                            addr_space="Shared")

    with tile.TileContext(nc) as tc:
        with tc.tile_pool(name="const", bufs=1) as const, \
             tc.tile_pool(name="qkp", bufs=1) as qkp, \
             tc.tile_pool(name="vp", bufs=1) as vp, \
             tc.tile_pool(name="work", bufs=4) as work, \
             tc.tile_pool(name="expp", bufs=4) as expp, \
             tc.tile_pool(name="gat", bufs=1) as gat, \
             tc.tile_pool(name="outp", bufs=3) as outp:

            # ---- constant loads -------------------------------------------
            zeros_t = const.tile([P, P], f32r, tag="zeros")
            nc.sync.dma_start(out=zeros_t, in_=zin[:, :])
            wqk = []
            xt = []
            for k in range(KD):
                t = const.tile([P, 384], f32r, tag=f"wqk{k}")
                nc.sync.dma_start(out=t, in_=w_qk[k * P:(k + 1) * P, :])
                wqk.append(t)
            for k in range(KD):
                t = const.tile([P, S], f32r, tag=f"xt{k}", name=f"xt{k}")
                xt.append(t)
            for half in range(2):
                for k in range(KD):
                    sl = slice(half * 1024, (half + 1) * 1024)
                    nc.scalar.dma_start(out=xt[k][:, sl], in_=xT[k * P:(k + 1) * P, sl])
            wv = []
            for k in range(KD):
                t = const.tile([P, 256], f32r, tag=f"wv{k}")
                nc.sync.dma_start(out=t, in_=w_v[k * P:(k + 1) * P, :])
                wv.append(t)
            bqk = []
            for m in range(3):
                t = const.tile([P, 1], f32, tag=f"bqk{m}")
                nc.sync.dma_start(out=t, in_=b_qk[m * P:(m + 1) * P, :])
                bqk.append(t)
            bv = const.tile([P, 256], f32, tag="bv")
            bv_bcast = bass.AP(tensor=b_v[:, :].tensor, offset=0,
                               ap=[[0, P], [1, 256]])
            nc.gpsimd.dma_start(out=bv, in_=bv_bcast)
            wo = []
            bo = []
            gix = []
            for k in range(KD):
                t = const.tile([P, D], f32r, tag=f"wo{k}")
                nc.gpsimd.dma_start(out=t, in_=w_o[k * P:(k + 1) * P, :])
                wo.append(t)
                t = const.tile([P, 1], f32, tag=f"bo{k}")
                nc.gpsimd.dma_start(out=t, in_=b_o[k * P:(k + 1) * P, :])
                bo.append(t)
                t = const.tile([P, 1], u32, tag=f"gix{k}")
                nc.gpsimd.dma_start(out=t, in_=gidx[k * P:(k + 1) * P, :])
                gix.append(t)

            # ---- attention -----------------------------------------------
            # Chunk = one [s_k 128, s_q 512] score block for one head.
            # Groups of 2 chunks share a 2-bank PSUM tile so one ACT exp
            # covers 1024 columns (amortizes the ~352-cycle ACT overhead).
            # Software-pipelined emission: mm_s(g+1) is emitted before
            # mm_c(g) so the PE never stalls behind the ACT.
            qkt = [qkp.tile([P, S], f32r, tag=f"qkt{m}", name=f"qkt{m}")
                   for m in range(3)]
            q2c = qkp.tile([64, S], f32r, tag="q2c")
            vsb = [vp.tile([P, 256], f32r, tag=f"v{s}", name=f"v{s}")
                   for s in range(SK)]

            def normalize(pc, nq, h):
                rec = work.tile([1, W], f32, tag="rec")
                nc.vector.reciprocal(rec[0:1, :], pc[64:65, :])
                rb = work.tile([64, W], f32, tag="rb")
                nc.gpsimd.partition_broadcast(rb, rec[:1, :])
                ctx = work.tile([64, W], f32r, tag="ctx")
                nc.vector.tensor_tensor(out=ctx, in0=pc[0:64, :], in1=rb,
                                        op=ALU.mult)
                nc.sync.dma_start(
                    out=cc_in[nq, h * HD:(h + 1) * HD, :],
                    in_=ctx)
                blk = nq * HPC + h
                nc.gpsimd.collective_compute(
                    "AllGather",
                    ALU.bypass,
                    ins=[cc_in[nq, h * HD:(h + 1) * HD, :]],
                    outs=[cc_all[blk * 512:(blk + 1) * 512, :]],
                    replica_groups=[list(range(N_CORES))],
                )

            # build group list: per nq, pair phase then solo phase
            groups = []
            for nq in range(NQ):
                for sk in range(SK):
                    groups.append({"nq": nq, "chunks": [(0, sk), (1, sk)],
                                   "last": False})
                for sk in range(0, SK, 2):
                    g = {"nq": nq, "chunks": [(2, sk), (2, sk + 1)],
                         "last": sk == SK - 2}
                    groups.append(g)

            pc_tiles = {}
            cnt = {}
            norm_done = {}
            ag_fired = set()

            def emit_mm_s(gi, grp):
                nq = grp["nq"]
                eps = psE.tile([P, 2 * W], f32, tag="ea" if gi % 2 == 0
                               else "eb", name=f"eps{gi}")
                for j, (h, sk) in enumerate(grp["chunks"]):
                    if h == 0:
                        lhsT = qkt[0][0:64, sk * P:(sk + 1) * P]
                        rhs = qkt[1][0:64, nq * W:(nq + 1) * W]
                    elif h == 1:
                        lhsT = qkt[0][64:128, sk * P:(sk + 1) * P]
                        rhs = qkt[1][64:128, nq * W:(nq + 1) * W]
                    else:
                        lhsT = qkt[2][0:64, sk * P:(sk + 1) * P]
                        rhs = q2c[:, nq * W:(nq + 1) * W]
                    nc.tensor.matmul(eps[:, j * W:(j + 1) * W], lhsT, rhs,
                                     start=True, stop=True)
                esb = expp.tile([P, 2 * W], f32r, tag="e", name=f"esb{gi}")
                nc.scalar.activation(esb, eps, AF.Exp)
                return esb

            def emit_mm_c(grp, esb):
                nq = grp["nq"]
                for j, (h, sk) in enumerate(grp["chunks"]):
                    key = (nq, h)
                    if key not in pc_tiles:
                        pc_tiles[key] = psC.tile([65, W], f32, tag="pc",
                                                 name=f"pc{nq}_{h}")
                        cnt[key] = 0
                    nc.tensor.matmul(
                        pc_tiles[key],
                        vsb[sk][:, h * 65:h * 65 + 65],
                        esb[:, j * W:(j + 1) * W],
                        start=(cnt[key] == 0), stop=(cnt[key] == SK - 1))
                    cnt[key] += 1
                    if cnt[key] == SK:
                        normalize(pc_tiles[key], nq, h)

            with tc.tile_pool(name="ps_proj", bufs=4, space="PSUM") as psP:

                def emit_qk_block(n):
                    for m in range(3):
                        ps = psP.tile([P, W], f32, tag="proj",
                                      name=f"psqk{n}_{m}")
                        first = n == 0 and m == 0
                        if first:
                            # zero-contribution matmuls: keep the PE busy
                            # while x/w DMAs land so HAM reaches full clock.
                            # They cover the whole [0:512] region with zeros
                            # so the real accumulation below lands on clean
                            # has_written state either way.
                            for d in range(24):
                                if d % 2 == 0:
                                    nc.tensor.matmul(
                                        ps[:, 0:384], zeros_t,
                                        wqk[0][:, :],
                                        start=(d == 0), stop=False,
                                        skip_group_check=True)
                                else:
                                    nc.tensor.matmul(
                                        ps[:, 384:512], zeros_t,
                                        wqk[1][:, 0:128],
                                        start=(d == 1), stop=False,
                                        skip_group_check=True)
                        for k in range(KD):
                            nc.tensor.matmul(
                                ps,
                                wqk[k][:, m * P:(m + 1) * P],
                                xt[k][:, n * W:(n + 1) * W],
                                start=(k == 0 and not first),
                                stop=(k == KD - 1),
                                skip_group_check=first)
                        nc.vector.tensor_scalar_add(
                            qkt[m][:, n * W:(n + 1) * W], ps, bqk[m])
                    nc.sync.dma_start(out=q2c[:, n * W:(n + 1) * W],
                                      in_=qkt[2][64:128, n * W:(n + 1) * W])

                def emit_v_block(n):
                    for s_ in range(4 * n, 4 * n + 4):
                        ps = psP.tile([P, W], f32, tag="proj",
                                      name=f"psv{s_}")
                        for k in range(KD):
                            nc.tensor.matmul(
                                ps[:, 0:256],
                                xt[k][:, s_ * P:(s_ + 1) * P],
                                wv[k],
                                start=(k == 0), stop=(k == KD - 1))
                        nc.vector.tensor_tensor(out=vsb[s_], in0=ps[:, 0:256],
                                                in1=bv, op=ALU.add)

                for n in range(NQ):
                    emit_qk_block(n)
                    emit_v_block(n)

            with tc.tile_pool(name="ps_e", bufs=1, space="PSUM") as psE, \
                 tc.tile_pool(name="ps_c", bufs=3, space="PSUM") as psC:
                prev = None
                for gi, grp in enumerate(groups):
                    esb = emit_mm_s(gi, grp)
                    if prev is not None:
                        emit_mm_c(prev[0], prev[1])
                    prev = (grp, esb)
                emit_mm_c(prev[0], prev[1])

            # ---- gather + output projection ------------------------------
            ctxg = []
            for k in range(KD):
                t = gat.tile([P, W], f32r, tag=f"ctxg{k}", name=f"ctxg{k}")
                nc.gpsimd.indirect_dma_start(
                    out=t,
                    out_offset=None,
                    in_=cc_all[:, :],
                    in_offset=bass.IndirectOffsetOnAxis(ap=gix[k][:, :1],
                                                        axis=0),
                )
                ctxg.append(t)
            with tc.tile_pool(name="ps_y", bufs=2, space="PSUM") as py:
                for m in range(KD):
                    ps = py.tile([P, W], f32)
                    if m == 0:
                        for d in range(16):
                            nc.tensor.matmul(
                                ps, zeros_t, wo[0][:, 0:W],
                                start=(d == 0), stop=False,
                                skip_group_check=True)
                    for k in range(KD):
                        nc.tensor.matmul(
                            ps,
                            wo[k][:, m * P:(m + 1) * P],
                            ctxg[k],
                            start=(k == 0 and m != 0),
                            stop=(k == KD - 1),
                            skip_group_check=(m == 0))
                    yt = outp.tile([P, W], f32r, tag="yt")
                    nc.vector.tensor_scalar_add(yt, ps, bo[m])
                    nc.sync.dma_start(out=out[m * P:(m + 1) * P, :], in_=yt)

    nc.compile()
    return nc


def _get_nc():
    if "nc" not in _CACHE:
        _install_profile_shim()
        _CACHE["nc"] = _build()
    return _CACHE["nc"]


def _make_in_maps(x, Wq, bq, Wk, bk, Wv, bv, Wo, bo):
    scale = np.float32(1.0 / np.sqrt(HD))
    f = np.float32
    x, Wq, bq, Wk, bk, Wv, bv, Wo, bo = [
        np.asarray(a, dtype=f) for a in (x, Wq, bq, Wk, bk, Wv, bv, Wo, bo)]

    in_maps = []
    for c in range(N_CORES):
        b = c // 4
        hs = (c % 4) * HPC
        q = c % 4
        hh = [hs, hs + 1, hs + 2]

        def wc(Wm, h):
            return Wm[:, h * HD:(h + 1) * HD]

        def bc(bm, h):
            return bm[h * HD:(h + 1) * HD]

        xTb = np.ascontiguousarray(x[b].T)
        w_qk = np.concatenate(
            [wc(Wk, hh[0]), wc(Wk, hh[1]),
             wc(Wq, hh[0]) * scale, wc(Wq, hh[1]) * scale,
             wc(Wk, hh[2]), wc(Wq, hh[2]) * scale], axis=1)
        b_qk = np.concatenate(
            [bc(bk, hh[0]), bc(bk, hh[1]),
             bc(bq, hh[0]) * scale, bc(bq, hh[1]) * scale,
             bc(bk, hh[2]), bc(bq, hh[2]) * scale])[:, None]
        w_v = np.zeros((D, 256), dtype=f)
        b_v = np.zeros((1, 256), dtype=f)
        for i, h in enumerate(hh):
            w_v[:, i * 65:i * 65 + HD] = wc(Wv, h)
            b_v[0, i * 65:i * 65 + HD] = bc(bv, h)
            b_v[0, i * 65 + HD] = 1.0
        i_feat = np.arange(D, dtype=np.uint32)
        hh = (i_feat % 192) // 64
        g = ((q * 3 + hh) * 512 + (4 * b + i_feat // 192) * 64
             + (i_feat % 64))
        in_maps.append({
            "xT": np.ascontiguousarray(xTb),
            "w_qk": np.ascontiguousarray(w_qk),
            "b_qk": np.ascontiguousarray(b_qk),
            "w_v": w_v,
            "b_v": b_v,
            "w_o": np.ascontiguousarray(Wo),
            "b_o": np.ascontiguousarray(bo[:, None]),
            "gidx": g.astype(np.uint32)[:, None],
            "zin": np.zeros((P, P), dtype=f),
        })
    return in_maps


def kernel(x, Wq, bq, Wk, bk, Wv, bv, Wo, bo, _trace=False):
    from concourse.bass_utils import run_bass_kernel_spmd

    nc = _get_nc()
    in_maps = _make_in_maps(x, Wq, bq, Wk, bk, Wv, bv, Wo, bo)
    res = run_bass_kernel_spmd(nc, in_maps, list(range(N_CORES)),
                               trace=_trace)
    _CACHE["last_results"] = res
    y = np.empty((B, S, D), dtype=np.float32)
    for c in range(N_CORES):
        b = c // 4
        q = c % 4
        y[b, q * W:(q + 1) * W, :] = res.results[c]["out"].T
    return y


# revision 15
# speedup vs baseline: 1.0664x; 1.0159x over previous
"""Multi-head attention (B=2, S=2048, D=768, H=12) on 8 Trainium2 NeuronCores.

Sequence sharding, no collectives: core c handles batch b=c//4 and query rows
[q*512, (q+1)*512) with q=c%4. Each core:
  1. Projects Q^T for its 512 query rows (all 12 heads), K^T for all 2048 keys,
     and V (sequence-major, with a ones-column per head for the softmax
     denominator). K/V work is replicated 4x across the batch group -- that
     cost hides in the PE slack of the ACT-bound attention phase.
  2. Per head: scores^T = K @ Q^T (head pairs run concurrently in PE
     row-groups), exp on ScalarE (scores are O(1): no max subtraction needed),
     ctx^T_aug = V_aug^T @ exp(scores^T) gives context + denominator in one
     pass; normalize.
  3. y^T = Wo^T @ ctx^T + bo, written out; host transposes/concats.

All matmul operands are float32r (TF32-like, full PE rate); fp32 accumulate.
K-projection chunks for later head-pairs are emitted inside the attention
pipeline so the PE absorbs them while the ACT (exp) is the bottleneck.
"""
import sys

if "/opt/trn_rl_repo" not in sys.path:
    sys.path.insert(0, "/opt/trn_rl_repo")

import numpy as np

B, S, D, H = 2, 2048, 768, 12
HD = 64
P = 128
N_CORES = 8
NP = 6           # head pairs
SK = 16          # s_k chunks of 128
KD = 6           # D chunks of 128
W = 512          # s_q width per core
VW = 780         # V free width: 12 heads x (64 + ones col)

_CACHE = {}


def _install_profile_shim():
    """run_bass_kernel_spmd(trace=True) needs antenv.axon_hooks; provide it."""
    import contextlib
    import ctypes
    import types

    if "antenv.axon_hooks" in sys.modules:
        return
    try:
        lib = ctypes.CDLL("/opt/axon/libaxon_pjrt.so")
    except OSError:
        return
    if not hasattr(lib, "axon_start_nrt_profile"):
        return
    lib.axon_start_nrt_profile.argtypes = [
        ctypes.POINTER(ctypes.c_int64),
        ctypes.c_size_t,
    ]
    lib.axon_start_nrt_profile.restype = ctypes.c_int64
    lib.axon_stop_nrt_profile.argtypes = [ctypes.c_char_p]
    lib.axon_stop_nrt_profile.restype = ctypes.c_int64

    @contextlib.contextmanager
    def _hook(output_dir, device_ids):
        import jax

        jax.devices()
        if device_ids:
            ids = (ctypes.c_int64 * len(device_ids))(*device_ids)
            rc = lib.axon_start_nrt_profile(ids, len(device_ids))
        else:
            rc = lib.axon_start_nrt_profile(None, 0)
        if rc != 0:
            raise RuntimeError(f"axon_start_nrt_profile rc={rc}")
        try:
            yield
        finally:
            n = lib.axon_stop_nrt_profile(str(output_dir).encode())
            if n < 0:
                raise RuntimeError(f"axon_stop_nrt_profile rc={n}")

    mod = types.ModuleType("antenv.axon_hooks")
    mod.get_axon_ntff_profile_hook = lambda: _hook
    mod.set_axon_ntff_profile_hook = lambda h: None
    sys.modules["antenv.axon_hooks"] = mod


def _build():
    import concourse.bass as bass
    from concourse import bacc
    import concourse.tile as tile
    import concourse.mybir as mybir

    f32r = mybir.dt.float32r
    f32 = mybir.dt.float32
    AF = mybir.ActivationFunctionType
    ALU = mybir.AluOpType

    nc = bacc.Bacc("TRN2", target_bir_lowering=False, debug=False,
                   num_devices=N_CORES)

    xT = nc.dram_tensor("xT", [D, S], f32r, kind="ExternalInput")
    xq = nc.dram_tensor("xq", [D, W], f32r, kind="ExternalInput")
    w_q = nc.dram_tensor("w_q", [D, D], f32r, kind="ExternalInput")
    b_q = nc.dram_tensor("b_q", [D, 1], f32, kind="ExternalInput")
    w_k = nc.dram_tensor("w_k", [D, D], f32r, kind="ExternalInput")
    b_k = nc.dram_tensor("b_k", [D, 1], f32, kind="ExternalInput")
    w_v = nc.dram_tensor("w_v", [D, VW], f32r, kind="ExternalInput")
    b_v = nc.dram_tensor("b_v", [1, VW], f32, kind="ExternalInput")
    w_o = nc.dram_tensor("w_o", [D, D], f32r, kind="ExternalInput")
    b_o = nc.dram_tensor("b_o", [D, 1], f32, kind="ExternalInput")
    zin = nc.dram_tensor("zin", [P, P], f32r, kind="ExternalInput")
    out = nc.dram_tensor("out", [D, W], f32r, kind="ExternalOutput")

    with tile.TileContext(nc) as tc:
        with tc.tile_pool(name="persist", bufs=1) as per, \
             tc.tile_pool(name="xblk", bufs=2) as xbp, \
             tc.tile_pool(name="work", bufs=2) as work, \
             tc.tile_pool(name="expp", bufs=3) as expp:

            # tiles that live through attention
            kt = [per.tile([P, S], f32r, tag=f"kt{m}", name=f"kt{m}")
                  for m in range(KD)]
            qt = [per.tile([P, W], f32r, tag=f"qt{m}", name=f"qt{m}")
                  for m in range(KD)]
            vsb = [per.tile([P, VW], f32r, tag=f"v{s}", name=f"v{s}")
                   for s in range(SK)]
            ctxm = [per.tile([P, W], f32r, tag=f"ctx{m}", name=f"ctx{m}")
                    for m in range(KD)]
            zeros_t = per.tile([P, P], f32r, tag="zeros")
            nc.sync.dma_start(out=zeros_t, in_=zin[:, :])
            bqt = []
            bkt = []
            bot = []
            for m in range(KD):
                t = per.tile([P, 1], f32, tag=f"bq{m}", name=f"bq{m}")
                nc.sync.dma_start(out=t, in_=b_q[m * P:(m + 1) * P, :])
                bqt.append(t)
                t = per.tile([P, 1], f32, tag=f"bk{m}", name=f"bk{m}")
                nc.sync.dma_start(out=t, in_=b_k[m * P:(m + 1) * P, :])
                bkt.append(t)
                t = per.tile([P, 1], f32, tag=f"bo{m}", name=f"bo{m}")
                nc.sync.dma_start(out=t, in_=b_o[m * P:(m + 1) * P, :])
                bot.append(t)
            bvt = per.tile([P, VW], f32, tag="bv")
            bv_bcast = bass.AP(tensor=b_v[:, :].tensor, offset=0,
                               ap=[[0, P], [1, VW]])
            nc.gpsimd.dma_start(out=bvt, in_=bv_bcast)

            def fetch_xblk(src_ap, eng):
                """Load a [768, 512] column block as 6 [128, 512] tiles."""
                ts = []
                for k in range(KD):
                    t = xbp.tile([P, W], f32r, tag=f"xb{k}",
                                 name=f"xb{k}")
                    eng.dma_start(out=t, in_=src_ap(k))
                    ts.append(t)
                return ts

            # ---- Q projection (with PE warm-up prologue) ------------------
            with tc.tile_pool(name="wqp", bufs=1) as wqp, \
                 tc.tile_pool(name="ps_q", bufs=3, space="PSUM") as psQ:
                wqt = []
                for k in range(KD):
                    t = wqp.tile([P, D], f32r, tag=f"wq{k}", name=f"wq{k}")
                    nc.sync.dma_start(out=t, in_=w_q[k * P:(k + 1) * P, :])
                    wqt.append(t)
                xqt = fetch_xblk(lambda k: xq[k * P:(k + 1) * P, :],
                                 nc.sync)
                for m in range(KD):
                    ps = psQ.tile([P, W], f32)
                    if m == 0:
                        # zero-contribution matmuls warm the PE/HAM while
                        # the x/w DMAs land; they cover the whole [0:512]
                        # region so has_written state is clean
                        for d in range(20):
                            nc.tensor.matmul(
                                ps, zeros_t, wqt[0][:, 0:W],
                                start=(d == 0), stop=False,
                                skip_group_check=True)
                    for k in range(KD):
                        nc.tensor.matmul(
                            ps,
                            wqt[k][:, m * P:(m + 1) * P],
                            xqt[k],
                            start=(k == 0 and m != 0),
                            stop=(k == KD - 1),
                            skip_group_check=(m == 0))
                    nc.vector.tensor_scalar_add(qt[m], ps, bqt[m])

            # ---- V projection (x streamed in 4 column blocks) ------------
            with tc.tile_pool(name="wvp", bufs=1) as wvp, \
                 tc.tile_pool(name="ps_v", bufs=3, space="PSUM") as psV:
                wvt = []
                for k in range(KD):
                    t = wvp.tile([P, VW], f32r, tag=f"wv{k}", name=f"wv{k}")
                    nc.scalar.dma_start(out=t, in_=w_v[k * P:(k + 1) * P, :])
                    wvt.append(t)
                for blk in range(4):
                    xb = fetch_xblk(
                        lambda k: xT[k * P:(k + 1) * P,
                                     blk * W:(blk + 1) * W],
                        nc.scalar)
                    for si in range(4):
                        s_ = blk * 4 + si
                        ps = psV.tile([P, VW], f32)
                        for k in range(KD):
                            nc.tensor.matmul(
                                ps[:, 0:W],
                                xb[k][:, si * P:(si + 1) * P],
                                wvt[k][:, 0:W],
                                start=(k == 0), stop=(k == KD - 1))
                            nc.tensor.matmul(
                                ps[:, W:VW],
                                xb[k][:, si * P:(si + 1) * P],
                                wvt[k][:, W:VW],
                                start=(k == 0), stop=(k == KD - 1))
                        nc.vector.tensor_tensor(out=vsb[s_], in0=ps,
                                                in1=bvt, op=ALU.add)

            # ---- K projection + attention, fused -------------------------
            # K-chunk m=0 is projected up front; chunk m=p+1 is emitted in
            # 4 slices inside pair p's attention groups so the PE absorbs
            # it under the ACT-bound exp pipeline.
            with tc.tile_pool(name="wkp", bufs=1) as wkp, \
                 tc.tile_pool(name="ps_k", bufs=1, space="PSUM") as psK, \
                 tc.tile_pool(name="ps_e", bufs=1, space="PSUM") as psE, \
                 tc.tile_pool(name="ps_c", bufs=3, space="PSUM") as psC:
                wkt = []
                for k in range(KD):
                    t = wkp.tile([P, D], f32r, tag=f"wk{k}", name=f"wk{k}")
                    nc.scalar.dma_start(out=t, in_=w_k[k * P:(k + 1) * P, :])
                    wkt.append(t)

                def emit_k_chunk(m, n):
                    xb = fetch_xblk(
                        lambda k: xT[k * P:(k + 1) * P, n * W:(n + 1) * W],
                        nc.sync)
                    ps = psK.tile([P, W], f32, tag="psk", name=f"psk{m}_{n}")
                    for k in range(KD):
                        nc.tensor.matmul(
                            ps,
                            wkt[k][:, m * P:(m + 1) * P],
                            xb[k],
                            start=(k == 0), stop=(k == KD - 1))
                    nc.vector.tensor_scalar_add(
                        kt[m][:, n * W:(n + 1) * W], ps, bkt[m])

                for n in range(4):
                    emit_k_chunk(0, n)

                def normalize(pc, h):
                    rec = work.tile([1, W], f32, tag="rec")
                    nc.vector.reciprocal(rec[0:1, :], pc[64:65, :])
                    rb = work.tile([64, W], f32, tag="rb")
                    nc.gpsimd.partition_broadcast(rb, rec[:1, :])
                    ctx = work.tile([64, W], f32r, tag="ctx")
                    nc.vector.tensor_tensor(out=ctx, in0=pc[0:64, :],
                                            in1=rb, op=ALU.mult)
                    # place head features at ctxm[h//2][(h%2)*64 ...] via
                    # SBUF->SBUF DMA (partition remap)
                    nc.sync.dma_start(
                        out=ctxm[h // 2][(h % 2) * HD:(h % 2 + 1) * HD, :],
                        in_=ctx)

                pc_tiles = {}
                cnt = {}

                def emit_mm_s(gi, p, sk):
                    # one group: both heads of pair p at s_k chunk sk
                    eps = psE.tile([P, 2 * W], f32,
                                   tag="ea" if gi % 2 == 0 else "eb",
                                   name=f"eps{gi}")
                    for j in range(2):
                        lo = j * HD
                        nc.tensor.matmul(
                            eps[:, j * W:(j + 1) * W],
                            kt[p][lo:lo + HD, sk * P:(sk + 1) * P],
                            qt[p][lo:lo + HD, :],
                            start=True, stop=True)
                    esb = expp.tile([P, 2 * W], f32r, tag="e",
                                    name=f"esb{gi}")
                    nc.scalar.activation(esb, eps, AF.Exp)
                    return esb

                def emit_mm_c(p, sk, esb):
                    for j in range(2):
                        h = 2 * p + j
                        if h not in pc_tiles:
                            pc_tiles[h] = psC.tile([65, W], f32, tag="pc",
                                                   name=f"pc{h}")
                            cnt[h] = 0
                        nc.tensor.matmul(
                            pc_tiles[h],
                            vsb[sk][:, h * 65:h * 65 + 65],
                            esb[:, j * W:(j + 1) * W],
                            start=(cnt[h] == 0), stop=(cnt[h] == SK - 1))
                        cnt[h] += 1
                        if cnt[h] == SK:
                            normalize(pc_tiles[h], h)

                groups = [(p, sk) for p in range(NP) for sk in range(SK)]
                prev = None
                for gi, (p, sk) in enumerate(groups):
                    # interleave next pair's K-projection in 4 slices
                    if p < NP - 1 and sk in (0, 4, 8, 12):
                        emit_k_chunk(p + 1, sk // 4)
                    esb = emit_mm_s(gi, p, sk)
                    if prev is not None:
                        emit_mm_c(prev[0], prev[1], prev[2])
                    prev = (p, sk, esb)
                emit_mm_c(prev[0], prev[1], prev[2])

            # ---- output projection ---------------------------------------
            with tc.tile_pool(name="wop", bufs=1) as wop, \
                 tc.tile_pool(name="outp", bufs=3) as outp, \
                 tc.tile_pool(name="ps_y", bufs=3, space="PSUM") as psY:
                wot = []
                for k in range(KD):
                    t = wop.tile([P, D], f32r, tag=f"wo{k}", name=f"wo{k}")
                    nc.gpsimd.dma_start(out=t, in_=w_o[k * P:(k + 1) * P, :])
                    wot.append(t)
                for m in range(KD):
                    ps = psY.tile([P, W], f32)
                    for k in range(KD):
                        nc.tensor.matmul(
                            ps,
                            wot[k][:, m * P:(m + 1) * P],
                            ctxm[k],
                            start=(k == 0), stop=(k == KD - 1))
                    yt = outp.tile([P, W], f32r, tag="yt")
                    nc.vector.tensor_scalar_add(yt, ps, bot[m])
                    nc.sync.dma_start(out=out[m * P:(m + 1) * P, :], in_=yt)

    nc.compile()
    return nc


def _get_nc():
    if "nc" not in _CACHE:
        _install_profile_shim()
        _CACHE["nc"] = _build()
    return _CACHE["nc"]


def _make_in_maps(x, Wq, bq, Wk, bk, Wv, bv, Wo, bo):
    scale = np.float32(1.0 / np.sqrt(HD))
    f = np.float32
    x, Wq, bq, Wk, bk, Wv, bv, Wo, bo = [
        np.asarray(a, dtype=f) for a in (x, Wq, bq, Wk, bk, Wv, bv, Wo, bo)]

    w_v = np.zeros((D, VW), dtype=f)
    b_v = np.zeros((1, VW), dtype=f)
    for h in range(H):
        w_v[:, h * 65:h * 65 + HD] = Wv[:, h * HD:(h + 1) * HD]
        b_v[0, h * 65:h * 65 + HD] = bv[h * HD:(h + 1) * HD]
        b_v[0, h * 65 + HD] = 1.0
    wq_s = np.ascontiguousarray(Wq * scale)
    bq_s = np.ascontiguousarray((bq * scale)[:, None])
    zeros = np.zeros((P, P), dtype=f)

    in_maps = []
    xTb = [np.ascontiguousarray(x[b].T) for b in range(B)]
    for c in range(N_CORES):
        b = c // 4
        q = c % 4
        in_maps.append({
            "xT": xTb[b],
            "xq": np.ascontiguousarray(xTb[b][:, q * W:(q + 1) * W]),
            "w_q": wq_s,
            "b_q": bq_s,
            "w_k": np.ascontiguousarray(Wk),
            "b_k": np.ascontiguousarray(bk[:, None]),
            "w_v": w_v,
            "b_v": b_v,
            "w_o": np.ascontiguousarray(Wo),
            "b_o": np.ascontiguousarray(bo[:, None]),
            "zin": zeros,
        })
    return in_maps


def kernel(x, Wq, bq, Wk, bk, Wv, bv, Wo, bo, _trace=False):
    from concourse.bass_utils import run_bass_kernel_spmd

    nc = _get_nc()
    in_maps = _make_in_maps(x, Wq, bq, Wk, bk, Wv, bv, Wo, bo)
    res = run_bass_kernel_spmd(nc, in_maps, list(range(N_CORES)),
                               trace=_trace)
    _CACHE["last_results"] = res
    y = np.empty((B, S, D), dtype=np.float32)
    for c in range(N_CORES):
        b = c // 4
        q = c % 4
        y[b, q * W:(q + 1) * W, :] = res.results[c]["out"].T
    return y


# revision 16
# speedup vs baseline: 1.1567x; 1.0847x over previous
"""Multi-head attention (B=2, S=2048, D=768, H=12) on 8 Trainium2 NeuronCores.

Sharding: core c handles batch b=c//4 and heads 3*(c%4) .. 3*(c%4)+2.
Each core:
  1. Projects Q,K (feature-major, transposed) and V (sequence-major, with an
     appended ones-column for the softmax denominator) for its 3 heads.
  2. Computes scores^T = K @ Q^T per head (contraction over head_dim=64, heads
     paired into PE row-groups), exp on ScalarE (scores are O(1), no max
     subtraction needed), then ctx^T_aug = V_aug^T @ exp(scores^T) which yields
     both the unnormalized context and the softmax denominator in one pass.
  3. Normalizes, writes local ctx^T [192, 2048] to DRAM.
  4. One 8-rank AllGather -> ctx^T for all heads/batches [1536, 2048].
  5. Indirect-gathers its (batch, s_q quarter) slice and computes the output
     projection y^T[:, q*512:(q+1)*512] = Wo^T @ ctx^T + bo.
Host assembles y[b, q*512:(q+1)*512, :] = out_c^T.

All matmul operands are float32r (TF32-like, full PE rate); accumulation fp32.
"""
import sys

if "/opt/trn_rl_repo" not in sys.path:
    sys.path.insert(0, "/opt/trn_rl_repo")

import numpy as np

B, S, D, H = 2, 2048, 768, 12
HD = 64
P = 128
N_CORES = 8
HPC = 3          # heads per core
NQ = 4           # s_q chunks of 512
SK = 16          # s_k chunks of 128
KD = 6           # D chunks of 128
W = 512          # working free-dim chunk

_CACHE = {}


def _install_profile_shim():
    """run_bass_kernel_spmd(trace=True) needs antenv.axon_hooks; provide it."""
    import contextlib
    import ctypes
    import types

    if "antenv.axon_hooks" in sys.modules:
        return
    try:
        lib = ctypes.CDLL("/opt/axon/libaxon_pjrt.so")
    except OSError:
        return
    if not hasattr(lib, "axon_start_nrt_profile"):
        return
    lib.axon_start_nrt_profile.argtypes = [
        ctypes.POINTER(ctypes.c_int64),
        ctypes.c_size_t,
    ]
    lib.axon_start_nrt_profile.restype = ctypes.c_int64
    lib.axon_stop_nrt_profile.argtypes = [ctypes.c_char_p]
    lib.axon_stop_nrt_profile.restype = ctypes.c_int64

    @contextlib.contextmanager
    def _hook(output_dir, device_ids):
        import jax

        jax.devices()
        if device_ids:
            ids = (ctypes.c_int64 * len(device_ids))(*device_ids)
            rc = lib.axon_start_nrt_profile(ids, len(device_ids))
        else:
            rc = lib.axon_start_nrt_profile(None, 0)
        if rc != 0:
            raise RuntimeError(f"axon_start_nrt_profile rc={rc}")
        try:
            yield
        finally:
            n = lib.axon_stop_nrt_profile(str(output_dir).encode())
            if n < 0:
                raise RuntimeError(f"axon_stop_nrt_profile rc={n}")

    mod = types.ModuleType("antenv.axon_hooks")
    mod.get_axon_ntff_profile_hook = lambda: _hook
    mod.set_axon_ntff_profile_hook = lambda h: None
    sys.modules["antenv.axon_hooks"] = mod


def _build():
    import concourse.bass as bass
    from concourse import bacc
    import concourse.tile as tile
    import concourse.mybir as mybir

    f32r = mybir.dt.float32r
    f32 = mybir.dt.float32
    u32 = mybir.dt.uint32
    AF = mybir.ActivationFunctionType
    ALU = mybir.AluOpType

    nc = bacc.Bacc("TRN2", target_bir_lowering=False, debug=False,
                   num_devices=N_CORES)

    xT = nc.dram_tensor("xT", [D, S], f32r, kind="ExternalInput")
    w_qk = nc.dram_tensor("w_qk", [D, 384], f32r, kind="ExternalInput")
    b_qk = nc.dram_tensor("b_qk", [384, 1], f32, kind="ExternalInput")
    w_v = nc.dram_tensor("w_v", [D, 256], f32r, kind="ExternalInput")
    b_v = nc.dram_tensor("b_v", [1, 256], f32, kind="ExternalInput")
    w_o = nc.dram_tensor("w_o", [D, D], f32r, kind="ExternalInput")
    b_o = nc.dram_tensor("b_o", [D, 1], f32, kind="ExternalInput")
    gidx = nc.dram_tensor("gidx", [D, 1], u32, kind="ExternalInput")
    out = nc.dram_tensor("out", [D, W], f32r, kind="ExternalOutput")

    cc_in = nc.dram_tensor("cc_in", [NQ, HPC * HD, W], f32r)
    cc_all = nc.dram_tensor("cc_all", [NQ * N_CORES * HPC * HD, W], f32r,
                            addr_space="Shared")

    with tile.TileContext(nc) as tc:
        with tc.tile_pool(name="const", bufs=1) as const, \
             tc.tile_pool(name="qkp", bufs=1) as qkp, \
             tc.tile_pool(name="vp", bufs=1) as vp, \
             tc.tile_pool(name="work", bufs=4) as work, \
             tc.tile_pool(name="expp", bufs=4) as expp, \
             tc.tile_pool(name="gat", bufs=1) as gat, \
             tc.tile_pool(name="outp", bufs=3) as outp:

            # ---- constant loads -------------------------------------------
            wqk = []
            xt = []
            for k in range(KD):
                t = const.tile([P, 384], f32r, tag=f"wqk{k}")
                nc.sync.dma_start(out=t, in_=w_qk[k * P:(k + 1) * P, :])
                wqk.append(t)
            for k in range(KD):
                t = const.tile([P, S], f32r, tag=f"xt{k}", name=f"xt{k}")
                xt.append(t)
            for k in range(KD):
                nc.sync.dma_start(out=xt[k][:, 0:1024],
                                  in_=xT[k * P:(k + 1) * P, 0:1024])
                nc.sync.dma_start(out=xt[k][:, 1024:2048],
                                  in_=xT[k * P:(k + 1) * P, 1024:2048])
            wv = []
            for k in range(KD):
                t = const.tile([P, 256], f32r, tag=f"wv{k}")
                nc.sync.dma_start(out=t, in_=w_v[k * P:(k + 1) * P, :])
                wv.append(t)
            bqk = []
            for m in range(3):
                t = const.tile([P, 1], f32, tag=f"bqk{m}")
                nc.sync.dma_start(out=t, in_=b_qk[m * P:(m + 1) * P, :])
                bqk.append(t)
            bv = const.tile([P, 256], f32, tag="bv")
            bv_bcast = bass.AP(tensor=b_v[:, :].tensor, offset=0,
                               ap=[[0, P], [1, 256]])
            nc.gpsimd.dma_start(out=bv, in_=bv_bcast)
            wo = []
            bo = []
            gix = []
            for k in range(KD):
                t = const.tile([P, D], f32r, tag=f"wo{k}")
                nc.sync.dma_start(out=t, in_=w_o[k * P:(k + 1) * P, :])
                wo.append(t)
                t = const.tile([P, 1], f32, tag=f"bo{k}")
                nc.sync.dma_start(out=t, in_=b_o[k * P:(k + 1) * P, :])
                bo.append(t)
                t = const.tile([P, 1], u32, tag=f"gix{k}")
                nc.sync.dma_start(out=t, in_=gidx[k * P:(k + 1) * P, :])
                gix.append(t)

            # ---- attention -----------------------------------------------
            # Chunk = one [s_k 128, s_q 512] score block for one head.
            # Groups of 2 chunks share a 2-bank PSUM tile so one ACT exp
            # covers 1024 columns (amortizes the ~352-cycle ACT overhead).
            # Software-pipelined emission: mm_s(g+1) is emitted before
            # mm_c(g) so the PE never stalls behind the ACT.
            qkt = [qkp.tile([P, S], f32r, tag=f"qkt{m}", name=f"qkt{m}")
                   for m in range(3)]
            q2c = qkp.tile([64, S], f32r, tag="q2c")
            vsb = [vp.tile([P, 256], f32r, tag=f"v{s}", name=f"v{s}")
                   for s in range(SK)]

            def normalize(pc, nq, h):
                rec = work.tile([1, W], f32, tag="rec")
                nc.vector.reciprocal(rec[0:1, :], pc[64:65, :])
                rb = work.tile([64, W], f32, tag="rb")
                nc.gpsimd.partition_broadcast(rb, rec[:1, :])
                ctx = work.tile([64, W], f32r, tag="ctx")
                nc.vector.tensor_tensor(out=ctx, in0=pc[0:64, :], in1=rb,
                                        op=ALU.mult)
                nc.sync.dma_start(
                    out=cc_in[nq, h * HD:(h + 1) * HD, :],
                    in_=ctx)
                norm_done.setdefault(nq, set()).add(h)
                if norm_done[nq] == {0, 1, 2}:
                    nc.gpsimd.collective_compute(
                        "AllGather",
                        ALU.bypass,
                        ins=[cc_in[nq]],
                        outs=[cc_all[nq * 1536:(nq + 1) * 1536, :]],
                        replica_groups=[list(range(N_CORES))],
                    )

            # build group list: per nq, pair phase then solo phase
            groups = []
            for nq in range(NQ):
                for sk in range(SK):
                    groups.append({"nq": nq, "chunks": [(0, sk), (1, sk)],
                                   "last": False})
                for sk in range(0, SK, 2):
                    g = {"nq": nq, "chunks": [(2, sk), (2, sk + 1)],
                         "last": sk == SK - 2}
                    groups.append(g)

            pc_tiles = {}
            cnt = {}
            norm_done = {}
            ag_fired = set()

            def emit_mm_s(gi, grp):
                nq = grp["nq"]
                eps = psE.tile([P, 2 * W], f32, tag="ea" if gi % 2 == 0
                               else "eb", name=f"eps{gi}")
                for j, (h, sk) in enumerate(grp["chunks"]):
                    if h == 0:
                        lhsT = qkt[0][0:64, sk * P:(sk + 1) * P]
                        rhs = qkt[1][0:64, nq * W:(nq + 1) * W]
                    elif h == 1:
                        lhsT = qkt[0][64:128, sk * P:(sk + 1) * P]
                        rhs = qkt[1][64:128, nq * W:(nq + 1) * W]
                    else:
                        lhsT = qkt[2][0:64, sk * P:(sk + 1) * P]
                        rhs = q2c[:, nq * W:(nq + 1) * W]
                    nc.tensor.matmul(eps[:, j * W:(j + 1) * W], lhsT, rhs,
                                     start=True, stop=True)
                esb = expp.tile([P, 2 * W], f32r, tag="e", name=f"esb{gi}")
                nc.scalar.activation(esb, eps, AF.Exp)
                return esb

            def emit_mm_c(grp, esb):
                nq = grp["nq"]
                for j, (h, sk) in enumerate(grp["chunks"]):
                    key = (nq, h)
                    if key not in pc_tiles:
                        pc_tiles[key] = psC.tile([65, W], f32, tag="pc",
                                                 name=f"pc{nq}_{h}")
                        cnt[key] = 0
                    nc.tensor.matmul(
                        pc_tiles[key],
                        vsb[sk][:, h * 65:h * 65 + 65],
                        esb[:, j * W:(j + 1) * W],
                        start=(cnt[key] == 0), stop=(cnt[key] == SK - 1))
                    cnt[key] += 1
                    if cnt[key] == SK:
                        normalize(pc_tiles[key], nq, h)

            with tc.tile_pool(name="ps_proj", bufs=4, space="PSUM") as psP:

                def emit_qk_block(n):
                    for m in range(3):
                        ps = psP.tile([P, W], f32, tag="proj",
                                      name=f"psqk{n}_{m}")
                        for k in range(KD):
                            nc.tensor.matmul(
                                ps,
                                wqk[k][:, m * P:(m + 1) * P],
                                xt[k][:, n * W:(n + 1) * W],
                                start=(k == 0), stop=(k == KD - 1))
                        nc.vector.tensor_scalar_add(
                            qkt[m][:, n * W:(n + 1) * W], ps, bqk[m])
                    nc.sync.dma_start(out=q2c[:, n * W:(n + 1) * W],
                                      in_=qkt[2][64:128, n * W:(n + 1) * W])

                def emit_v_block(n):
                    for s_ in range(4 * n, 4 * n + 4):
                        ps = psP.tile([P, W], f32, tag="proj",
                                      name=f"psv{s_}")
                        for k in range(KD):
                            nc.tensor.matmul(
                                ps[:, 0:256],
                                xt[k][:, s_ * P:(s_ + 1) * P],
                                wv[k],
                                start=(k == 0), stop=(k == KD - 1))
                        nc.vector.tensor_tensor(out=vsb[s_], in0=ps[:, 0:256],
                                                in1=bv, op=ALU.add)

                for n in range(NQ):
                    emit_qk_block(n)
                    emit_v_block(n)

            with tc.tile_pool(name="ps_e", bufs=1, space="PSUM") as psE, \
                 tc.tile_pool(name="ps_c", bufs=3, space="PSUM") as psC:
                prev = None
                for gi, grp in enumerate(groups):
                    esb = emit_mm_s(gi, grp)
                    if prev is not None:
                        emit_mm_c(prev[0], prev[1])
                    prev = (grp, esb)
                emit_mm_c(prev[0], prev[1])

            # ---- gather + output projection ------------------------------
            ctxg = []
            for k in range(KD):
                t = gat.tile([P, W], f32r, tag=f"ctxg{k}", name=f"ctxg{k}")
                nc.gpsimd.indirect_dma_start(
                    out=t,
                    out_offset=None,
                    in_=cc_all[:, :],
                    in_offset=bass.IndirectOffsetOnAxis(ap=gix[k][:, :1],
                                                        axis=0),
                )
                ctxg.append(t)
            with tc.tile_pool(name="ps_y", bufs=2, space="PSUM") as py:
                for m in range(KD):
                    ps = py.tile([P, W], f32)
                    for k in range(KD):
                        nc.tensor.matmul(
                            ps,
                            wo[k][:, m * P:(m + 1) * P],
                            ctxg[k],
                            start=(k == 0), stop=(k == KD - 1))
                    yt = outp.tile([P, W], f32r, tag="yt")
                    nc.vector.tensor_scalar_add(yt, ps, bo[m])
                    nc.sync.dma_start(out=out[m * P:(m + 1) * P, :], in_=yt)

    nc.compile()
    return nc


def _get_nc():
    if "nc" not in _CACHE:
        _install_profile_shim()
        _CACHE["nc"] = _build()
    return _CACHE["nc"]


def _make_in_maps(x, Wq, bq, Wk, bk, Wv, bv, Wo, bo):
    scale = np.float32(1.0 / np.sqrt(HD))
    f = np.float32
    x, Wq, bq, Wk, bk, Wv, bv, Wo, bo = [
        np.asarray(a, dtype=f) for a in (x, Wq, bq, Wk, bk, Wv, bv, Wo, bo)]

    in_maps = []
    for c in range(N_CORES):
        b = c // 4
        hs = (c % 4) * HPC
        q = c % 4
        hh = [hs, hs + 1, hs + 2]

        def wc(Wm, h):
            return Wm[:, h * HD:(h + 1) * HD]

        def bc(bm, h):
            return bm[h * HD:(h + 1) * HD]

        xTb = np.ascontiguousarray(x[b].T)
        w_qk = np.concatenate(
            [wc(Wk, hh[0]), wc(Wk, hh[1]),
             wc(Wq, hh[0]) * scale, wc(Wq, hh[1]) * scale,
             wc(Wk, hh[2]), wc(Wq, hh[2]) * scale], axis=1)
        b_qk = np.concatenate(
            [bc(bk, hh[0]), bc(bk, hh[1]),
             bc(bq, hh[0]) * scale, bc(bq, hh[1]) * scale,
             bc(bk, hh[2]), bc(bq, hh[2]) * scale])[:, None]
        w_v = np.zeros((D, 256), dtype=f)
        b_v = np.zeros((1, 256), dtype=f)
        for i, h in enumerate(hh):
            w_v[:, i * 65:i * 65 + HD] = wc(Wv, h)
            b_v[0, i * 65:i * 65 + HD] = bc(bv, h)
            b_v[0, i * 65 + HD] = 1.0
        i_feat = np.arange(D, dtype=np.uint32)
        g = q * 1536 + (4 * b + i_feat // 192) * 192 + (i_feat % 192)
        in_maps.append({
            "xT": np.ascontiguousarray(xTb),
            "w_qk": np.ascontiguousarray(w_qk),
            "b_qk": np.ascontiguousarray(b_qk),
            "w_v": w_v,
            "b_v": b_v,
            "w_o": np.ascontiguousarray(Wo),
            "b_o": np.ascontiguousarray(bo[:, None]),
            "gidx": g.astype(np.uint32)[:, None],
        })
    return in_maps


def kernel(x, Wq, bq, Wk, bk, Wv, bv, Wo, bo, _trace=False):
    from concourse.bass_utils import run_bass_kernel_spmd

    nc = _get_nc()
    in_maps = _make_in_maps(x, Wq, bq, Wk, bk, Wv, bv, Wo, bo)
    res = run_bass_kernel_spmd(nc, in_maps, list(range(N_CORES)),
                               trace=_trace)
    _CACHE["last_results"] = res
    y = np.empty((B, S, D), dtype=np.float32)
    for c in range(N_CORES):
        b = c // 4
        q = c % 4
        y[b, q * W:(q + 1) * W, :] = res.results[c]["out"].T
    return y


# revision 17
# speedup vs baseline: 1.1943x; 1.0325x over previous
"""Multi-head attention (B=2, S=2048, D=768, H=12) on 8 Trainium2 NeuronCores.

Sharding: core c handles batch b=c//4 and heads 3*(c%4) .. 3*(c%4)+2.
Each core:
  1. Projects Q,K (feature-major, transposed) and V (sequence-major, with an
     appended ones-column for the softmax denominator) for its 3 heads.
  2. Computes scores^T = K @ Q^T per head (contraction over head_dim=64, heads
     paired into PE row-groups), exp on ScalarE (scores are O(1), no max
     subtraction needed), then ctx^T_aug = V_aug^T @ exp(scores^T) which yields
     both the unnormalized context and the softmax denominator in one pass.
  3. Normalizes, writes local ctx^T [192, 2048] to DRAM.
  4. One 8-rank AllGather -> ctx^T for all heads/batches [1536, 2048].
  5. Indirect-gathers its (batch, s_q quarter) slice and computes the output
     projection y^T[:, q*512:(q+1)*512] = Wo^T @ ctx^T + bo.
Host assembles y[b, q*512:(q+1)*512, :] = out_c^T.

All matmul operands are float32r (TF32-like, full PE rate); accumulation fp32.
"""
import sys

if "/opt/trn_rl_repo" not in sys.path:
    sys.path.insert(0, "/opt/trn_rl_repo")

import numpy as np

B, S, D, H = 2, 2048, 768, 12
HD = 64
P = 128
N_CORES = 8
HPC = 3          # heads per core
NQ = 4           # s_q chunks of 512
SK = 16          # s_k chunks of 128
KD = 6           # D chunks of 128
W = 512          # working free-dim chunk

_CACHE = {}


def _install_profile_shim():
    """run_bass_kernel_spmd(trace=True) needs antenv.axon_hooks; provide it."""
    import contextlib
    import ctypes
    import types

    if "antenv.axon_hooks" in sys.modules:
        return
    try:
        lib = ctypes.CDLL("/opt/axon/libaxon_pjrt.so")
    except OSError:
        return
    if not hasattr(lib, "axon_start_nrt_profile"):
        return
    lib.axon_start_nrt_profile.argtypes = [
        ctypes.POINTER(ctypes.c_int64),
        ctypes.c_size_t,
    ]
    lib.axon_start_nrt_profile.restype = ctypes.c_int64
    lib.axon_stop_nrt_profile.argtypes = [ctypes.c_char_p]
    lib.axon_stop_nrt_profile.restype = ctypes.c_int64

    @contextlib.contextmanager
    def _hook(output_dir, device_ids):
        import jax

        jax.devices()
        if device_ids:
            ids = (ctypes.c_int64 * len(device_ids))(*device_ids)
            rc = lib.axon_start_nrt_profile(ids, len(device_ids))
        else:
            rc = lib.axon_start_nrt_profile(None, 0)
        if rc != 0:
            raise RuntimeError(f"axon_start_nrt_profile rc={rc}")
        try:
            yield
        finally:
            n = lib.axon_stop_nrt_profile(str(output_dir).encode())
            if n < 0:
                raise RuntimeError(f"axon_stop_nrt_profile rc={n}")

    mod = types.ModuleType("antenv.axon_hooks")
    mod.get_axon_ntff_profile_hook = lambda: _hook
    mod.set_axon_ntff_profile_hook = lambda h: None
    sys.modules["antenv.axon_hooks"] = mod


def _build():
    import concourse.bass as bass
    from concourse import bacc
    import concourse.tile as tile
    import concourse.mybir as mybir

    f32r = mybir.dt.float32r
    f32 = mybir.dt.float32
    u32 = mybir.dt.uint32
    AF = mybir.ActivationFunctionType
    ALU = mybir.AluOpType

    nc = bacc.Bacc("TRN2", target_bir_lowering=False, debug=False,
                   num_devices=N_CORES)

    xT = nc.dram_tensor("xT", [D, S], f32r, kind="ExternalInput")
    w_qk = nc.dram_tensor("w_qk", [D, 384], f32r, kind="ExternalInput")
    b_qk = nc.dram_tensor("b_qk", [384, 1], f32, kind="ExternalInput")
    w_v = nc.dram_tensor("w_v", [D, 256], f32r, kind="ExternalInput")
    b_v = nc.dram_tensor("b_v", [1, 256], f32, kind="ExternalInput")
    w_o = nc.dram_tensor("w_o", [D, D], f32r, kind="ExternalInput")
    b_o = nc.dram_tensor("b_o", [D, 1], f32, kind="ExternalInput")
    gidx = nc.dram_tensor("gidx", [D, 1], u32, kind="ExternalInput")
    zin = nc.dram_tensor("zin", [P, P], f32r, kind="ExternalInput")
    out = nc.dram_tensor("out", [D, W], f32r, kind="ExternalOutput")

    cc_in = nc.dram_tensor("cc_in", [NQ, HPC * HD, W], f32r)
    cc_all = nc.dram_tensor("cc_all", [NQ * N_CORES * HPC * HD, W], f32r,
                            addr_space="Shared")

    with tile.TileContext(nc) as tc:
        with tc.tile_pool(name="const", bufs=1) as const, \
             tc.tile_pool(name="qkp", bufs=1) as qkp, \
             tc.tile_pool(name="vp", bufs=1) as vp, \
             tc.tile_pool(name="work", bufs=4) as work, \
             tc.tile_pool(name="expp", bufs=4) as expp, \
             tc.tile_pool(name="gat", bufs=1) as gat, \
             tc.tile_pool(name="outp", bufs=3) as outp:

            # ---- constant loads -------------------------------------------
            zeros_t = const.tile([P, P], f32r, tag="zeros")
            nc.sync.dma_start(out=zeros_t, in_=zin[:, :])
            wqk = []
            xt = []
            for k in range(KD):
                t = const.tile([P, 384], f32r, tag=f"wqk{k}")
                nc.sync.dma_start(out=t, in_=w_qk[k * P:(k + 1) * P, :])
                wqk.append(t)
            for k in range(KD):
                t = const.tile([P, S], f32r, tag=f"xt{k}", name=f"xt{k}")
                xt.append(t)
            for k in range(KD):
                nc.sync.dma_start(out=xt[k][:, 0:1024],
                                  in_=xT[k * P:(k + 1) * P, 0:1024])
                nc.sync.dma_start(out=xt[k][:, 1024:2048],
                                  in_=xT[k * P:(k + 1) * P, 1024:2048])
            wv = []
            for k in range(KD):
                t = const.tile([P, 256], f32r, tag=f"wv{k}")
                nc.sync.dma_start(out=t, in_=w_v[k * P:(k + 1) * P, :])
                wv.append(t)
            bqk = []
            for m in range(3):
                t = const.tile([P, 1], f32, tag=f"bqk{m}")
                nc.sync.dma_start(out=t, in_=b_qk[m * P:(m + 1) * P, :])
                bqk.append(t)
            bv = const.tile([P, 256], f32, tag="bv")
            bv_bcast = bass.AP(tensor=b_v[:, :].tensor, offset=0,
                               ap=[[0, P], [1, 256]])
            nc.gpsimd.dma_start(out=bv, in_=bv_bcast)
            wo = []
            bo = []
            gix = []
            for k in range(KD):
                t = const.tile([P, D], f32r, tag=f"wo{k}")
                nc.sync.dma_start(out=t, in_=w_o[k * P:(k + 1) * P, :])
                wo.append(t)
                t = const.tile([P, 1], f32, tag=f"bo{k}")
                nc.sync.dma_start(out=t, in_=b_o[k * P:(k + 1) * P, :])
                bo.append(t)
                t = const.tile([P, 1], u32, tag=f"gix{k}")
                nc.sync.dma_start(out=t, in_=gidx[k * P:(k + 1) * P, :])
                gix.append(t)

            # ---- attention -----------------------------------------------
            # Chunk = one [s_k 128, s_q 512] score block for one head.
            # Groups of 2 chunks share a 2-bank PSUM tile so one ACT exp
            # covers 1024 columns (amortizes the ~352-cycle ACT overhead).
            # Software-pipelined emission: mm_s(g+1) is emitted before
            # mm_c(g) so the PE never stalls behind the ACT.
            qkt = [qkp.tile([P, S], f32r, tag=f"qkt{m}", name=f"qkt{m}")
                   for m in range(3)]
            q2c = qkp.tile([64, S], f32r, tag="q2c")
            vsb = [vp.tile([P, 256], f32r, tag=f"v{s}", name=f"v{s}")
                   for s in range(SK)]

            def normalize(pc, nq, h):
                rec = work.tile([1, W], f32, tag="rec")
                nc.vector.reciprocal(rec[0:1, :], pc[64:65, :])
                rb = work.tile([64, W], f32, tag="rb")
                nc.gpsimd.partition_broadcast(rb, rec[:1, :])
                ctx = work.tile([64, W], f32r, tag="ctx")
                nc.vector.tensor_tensor(out=ctx, in0=pc[0:64, :], in1=rb,
                                        op=ALU.mult)
                nc.sync.dma_start(
                    out=cc_in[nq, h * HD:(h + 1) * HD, :],
                    in_=ctx)
                norm_done.setdefault(nq, set()).add(h)
                rg = [list(range(N_CORES))]
                if nq < 3:
                    if norm_done[nq] == {0, 1, 2}:
                        nc.gpsimd.collective_compute(
                            "AllGather", ALU.bypass,
                            ins=[cc_in[nq]],
                            outs=[cc_all[nq * 1536:(nq + 1) * 1536, :]],
                            replica_groups=rg)
                else:
                    # last quarter: gather the head pair as soon as it is
                    # done, leaving only the 64-row solo piece for the tail
                    if norm_done[nq] >= {0, 1} and "agA" not in norm_done:
                        norm_done["agA"] = True
                        nc.gpsimd.collective_compute(
                            "AllGather", ALU.bypass,
                            ins=[cc_in[3, 0:128, :]],
                            outs=[cc_all[4608:5632, :]],
                            replica_groups=rg)
                    if h == 2:
                        nc.gpsimd.collective_compute(
                            "AllGather", ALU.bypass,
                            ins=[cc_in[3, 128:192, :]],
                            outs=[cc_all[5632:6144, :]],
                            replica_groups=rg)

            # build group list: per nq, pair phase then solo phase
            groups = []
            for nq in range(NQ):
                for sk in range(SK):
                    groups.append({"nq": nq, "chunks": [(0, sk), (1, sk)],
                                   "last": False})
                for sk in range(0, SK, 2):
                    g = {"nq": nq, "chunks": [(2, sk), (2, sk + 1)],
                         "last": sk == SK - 2}
                    groups.append(g)

            pc_tiles = {}
            cnt = {}
            norm_done = {}
            ag_fired = set()

            def emit_mm_s(gi, grp):
                nq = grp["nq"]
                eps = psE.tile([P, 2 * W], f32, tag="ea" if gi % 2 == 0
                               else "eb", name=f"eps{gi}")
                for j, (h, sk) in enumerate(grp["chunks"]):
                    if h == 0:
                        lhsT = qkt[0][0:64, sk * P:(sk + 1) * P]
                        rhs = qkt[1][0:64, nq * W:(nq + 1) * W]
                    elif h == 1:
                        lhsT = qkt[0][64:128, sk * P:(sk + 1) * P]
                        rhs = qkt[1][64:128, nq * W:(nq + 1) * W]
                    else:
                        lhsT = qkt[2][0:64, sk * P:(sk + 1) * P]
                        rhs = q2c[:, nq * W:(nq + 1) * W]
                    nc.tensor.matmul(eps[:, j * W:(j + 1) * W], lhsT, rhs,
                                     start=True, stop=True)
                esb = expp.tile([P, 2 * W], f32r, tag="e", name=f"esb{gi}")
                nc.scalar.activation(esb, eps, AF.Exp)
                return esb

            def emit_mm_c(grp, esb):
                nq = grp["nq"]
                for j, (h, sk) in enumerate(grp["chunks"]):
                    key = (nq, h)
                    if key not in pc_tiles:
                        pc_tiles[key] = psC.tile([65, W], f32, tag="pc",
                                                 name=f"pc{nq}_{h}")
                        cnt[key] = 0
                    nc.tensor.matmul(
                        pc_tiles[key],
                        vsb[sk][:, h * 65:h * 65 + 65],
                        esb[:, j * W:(j + 1) * W],
                        start=(cnt[key] == 0), stop=(cnt[key] == SK - 1))
                    cnt[key] += 1
                    if cnt[key] == SK:
                        normalize(pc_tiles[key], nq, h)

            with tc.tile_pool(name="ps_proj", bufs=4, space="PSUM") as psP:

                def emit_qk_block(n):
                    for m in range(3):
                        ps = psP.tile([P, W], f32, tag="proj",
                                      name=f"psqk{n}_{m}")
                        first = n == 0 and m == 0
                        if first:
                            # zero-contribution warmup: keeps the PE busy
                            # while x DMAs land so HAM reaches 2.4GHz; the
                            # two regions cover [0:512] so has_written is
                            # clean for the real accumulation below
                            for d in range(24):
                                if d % 2 == 0:
                                    nc.tensor.matmul(
                                        ps[:, 0:384], zeros_t, wqk[0][:, :],
                                        start=(d == 0), stop=False,
                                        skip_group_check=True)
                                else:
                                    nc.tensor.matmul(
                                        ps[:, 384:512], zeros_t,
                                        wqk[1][:, 0:128],
                                        start=(d == 1), stop=False,
                                        skip_group_check=True)
                        for k in range(KD):
                            nc.tensor.matmul(
                                ps,
                                wqk[k][:, m * P:(m + 1) * P],
                                xt[k][:, n * W:(n + 1) * W],
                                start=(k == 0 and not first),
                                stop=(k == KD - 1),
                                skip_group_check=first)
                        nc.vector.tensor_scalar_add(
                            qkt[m][:, n * W:(n + 1) * W], ps, bqk[m])
                    nc.sync.dma_start(out=q2c[:, n * W:(n + 1) * W],
                                      in_=qkt[2][64:128, n * W:(n + 1) * W])

                def emit_v_block(n):
                    for s_ in range(4 * n, 4 * n + 4):
                        ps = psP.tile([P, W], f32, tag="proj",
                                      name=f"psv{s_}")
                        for k in range(KD):
                            nc.tensor.matmul(
                                ps[:, 0:256],
                                xt[k][:, s_ * P:(s_ + 1) * P],
                                wv[k],
                                start=(k == 0), stop=(k == KD - 1))
                        nc.vector.tensor_tensor(out=vsb[s_], in0=ps[:, 0:256],
                                                in1=bv, op=ALU.add)

                for n in range(NQ):
                    emit_qk_block(n)
                    emit_v_block(n)

            with tc.tile_pool(name="ps_e", bufs=1, space="PSUM") as psE, \
                 tc.tile_pool(name="ps_c", bufs=3, space="PSUM") as psC:
                prev = None
                for gi, grp in enumerate(groups):
                    esb = emit_mm_s(gi, grp)
                    if prev is not None:
                        emit_mm_c(prev[0], prev[1])
                    prev = (grp, esb)
                emit_mm_c(prev[0], prev[1])

            # ---- gather + output projection ------------------------------
            ctxg = []
            for k in range(KD):
                t = gat.tile([P, W], f32r, tag=f"ctxg{k}", name=f"ctxg{k}")
                nc.gpsimd.indirect_dma_start(
                    out=t,
                    out_offset=None,
                    in_=cc_all[:, :],
                    in_offset=bass.IndirectOffsetOnAxis(ap=gix[k][:, :1],
                                                        axis=0),
                )
                ctxg.append(t)
            with tc.tile_pool(name="ps_y", bufs=2, space="PSUM") as py:
                for m in range(KD):
                    ps = py.tile([P, W], f32)
                    if m == 0:
                        # warm the PE during the AllGather wait so the
                        # output projection runs at full clock
                        for d in range(16):
                            nc.tensor.matmul(
                                ps, zeros_t, wo[0][:, 0:W],
                                start=(d == 0), stop=False,
                                skip_group_check=True)
                    for k in range(KD):
                        nc.tensor.matmul(
                            ps,
                            wo[k][:, m * P:(m + 1) * P],
                            ctxg[k],
                            start=(k == 0 and m != 0),
                            stop=(k == KD - 1),
                            skip_group_check=(m == 0))
                    yt = outp.tile([P, W], f32r, tag="yt")
                    nc.vector.tensor_scalar_add(yt, ps, bo[m])
                    nc.sync.dma_start(out=out[m * P:(m + 1) * P, :], in_=yt)

    nc.compile()
    return nc


def _get_nc():
    if "nc" not in _CACHE:
        _install_profile_shim()
        _CACHE["nc"] = _build()
    return _CACHE["nc"]


def _make_in_maps(x, Wq, bq, Wk, bk, Wv, bv, Wo, bo):
    scale = np.float32(1.0 / np.sqrt(HD))
    f = np.float32
    x, Wq, bq, Wk, bk, Wv, bv, Wo, bo = [
        np.asarray(a, dtype=f) for a in (x, Wq, bq, Wk, bk, Wv, bv, Wo, bo)]

    in_maps = []
    for c in range(N_CORES):
        b = c // 4
        hs = (c % 4) * HPC
        q = c % 4
        hh = [hs, hs + 1, hs + 2]

        def wc(Wm, h):
            return Wm[:, h * HD:(h + 1) * HD]

        def bc(bm, h):
            return bm[h * HD:(h + 1) * HD]

        xTb = np.ascontiguousarray(x[b].T)
        w_qk = np.concatenate(
            [wc(Wk, hh[0]), wc(Wk, hh[1]),
             wc(Wq, hh[0]) * scale, wc(Wq, hh[1]) * scale,
             wc(Wk, hh[2]), wc(Wq, hh[2]) * scale], axis=1)
        b_qk = np.concatenate(
            [bc(bk, hh[0]), bc(bk, hh[1]),
             bc(bq, hh[0]) * scale, bc(bq, hh[1]) * scale,
             bc(bk, hh[2]), bc(bq, hh[2]) * scale])[:, None]
        w_v = np.zeros((D, 256), dtype=f)
        b_v = np.zeros((1, 256), dtype=f)
        for i, h in enumerate(hh):
            w_v[:, i * 65:i * 65 + HD] = wc(Wv, h)
            b_v[0, i * 65:i * 65 + HD] = bc(bv, h)
            b_v[0, i * 65 + HD] = 1.0
        i_feat = np.arange(D, dtype=np.uint32)
        r_g = 4 * b + i_feat // 192
        f_loc = i_feat % 192
        if q < 3:
            g = q * 1536 + r_g * 192 + f_loc
        else:
            g = np.where(f_loc < 128,
                         4608 + r_g * 128 + f_loc,
                         5632 + r_g * 64 + (f_loc - 128))
        in_maps.append({
            "xT": np.ascontiguousarray(xTb),
            "w_qk": np.ascontiguousarray(w_qk),
            "b_qk": np.ascontiguousarray(b_qk),
            "w_v": w_v,
            "b_v": b_v,
            "w_o": np.ascontiguousarray(Wo),
            "b_o": np.ascontiguousarray(bo[:, None]),
            "gidx": g.astype(np.uint32)[:, None],
            "zin": np.zeros((P, P), dtype=f),
        })
    return in_maps


def kernel(x, Wq, bq, Wk, bk, Wv, bv, Wo, bo, _trace=False):
    from concourse.bass_utils import run_bass_kernel_spmd

    nc = _get_nc()
    in_maps = _make_in_maps(x, Wq, bq, Wk, bk, Wv, bv, Wo, bo)
    res = run_bass_kernel_spmd(nc, in_maps, list(range(N_CORES)),
                               trace=_trace)
    _CACHE["last_results"] = res
    y = np.empty((B, S, D), dtype=np.float32)
    for c in range(N_CORES):
        b = c // 4
        q = c % 4
        y[b, q * W:(q + 1) * W, :] = res.results[c]["out"].T
    return y


# revision 18
# speedup vs baseline: 1.3067x; 1.0940x over previous
"""Multi-head attention (B=2, S=2048, D=768, H=12) on 8 Trainium2 NeuronCores.

Sharding: core c handles batch b=c//4 and heads 3*(c%4) .. 3*(c%4)+2.
Each core:
  1. Projects Q,K (feature-major, transposed) and V (sequence-major, with an
     appended ones-column for the softmax denominator) for its 3 heads.
  2. Computes scores^T = K @ Q^T per head (contraction over head_dim=64, heads
     paired into PE row-groups), exp on ScalarE (scores are O(1), no max
     subtraction needed), then ctx^T_aug = V_aug^T @ exp(scores^T) which yields
     both the unnormalized context and the softmax denominator in one pass.
  3. Normalizes, writes local ctx^T [192, 2048] to DRAM.
  4. One 8-rank AllGather -> ctx^T for all heads/batches [1536, 2048].
  5. Indirect-gathers its (batch, s_q quarter) slice and computes the output
     projection y^T[:, q*512:(q+1)*512] = Wo^T @ ctx^T + bo.
Host assembles y[b, q*512:(q+1)*512, :] = out_c^T.

All matmul operands are float32r (TF32-like, full PE rate); accumulation fp32.
"""
import sys

if "/opt/trn_rl_repo" not in sys.path:
    sys.path.insert(0, "/opt/trn_rl_repo")

import numpy as np

B, S, D, H = 2, 2048, 768, 12
HD = 64
P = 128
N_CORES = 8
HPC = 3          # heads per core
NQ = 4           # s_q chunks of 512
SK = 16          # s_k chunks of 128
KD = 6           # D chunks of 128
W = 512          # working free-dim chunk

_CACHE = {}


def _install_profile_shim():
    """run_bass_kernel_spmd(trace=True) needs antenv.axon_hooks; provide it."""
    import contextlib
    import ctypes
    import types

    if "antenv.axon_hooks" in sys.modules:
        return
    try:
        lib = ctypes.CDLL("/opt/axon/libaxon_pjrt.so")
    except OSError:
        return
    if not hasattr(lib, "axon_start_nrt_profile"):
        return
    lib.axon_start_nrt_profile.argtypes = [
        ctypes.POINTER(ctypes.c_int64),
        ctypes.c_size_t,
    ]
    lib.axon_start_nrt_profile.restype = ctypes.c_int64
    lib.axon_stop_nrt_profile.argtypes = [ctypes.c_char_p]
    lib.axon_stop_nrt_profile.restype = ctypes.c_int64

    @contextlib.contextmanager
    def _hook(output_dir, device_ids):
        import jax

        jax.devices()
        if device_ids:
            ids = (ctypes.c_int64 * len(device_ids))(*device_ids)
            rc = lib.axon_start_nrt_profile(ids, len(device_ids))
        else:
            rc = lib.axon_start_nrt_profile(None, 0)
        if rc != 0:
            raise RuntimeError(f"axon_start_nrt_profile rc={rc}")
        try:
            yield
        finally:
            n = lib.axon_stop_nrt_profile(str(output_dir).encode())
            if n < 0:
                raise RuntimeError(f"axon_stop_nrt_profile rc={n}")

    mod = types.ModuleType("antenv.axon_hooks")
    mod.get_axon_ntff_profile_hook = lambda: _hook
    mod.set_axon_ntff_profile_hook = lambda h: None
    sys.modules["antenv.axon_hooks"] = mod


def _build():
    import concourse.bass as bass
    from concourse import bacc
    import concourse.tile as tile
    import concourse.mybir as mybir

    f32r = mybir.dt.float32r
    f32 = mybir.dt.float32
    u32 = mybir.dt.uint32
    AF = mybir.ActivationFunctionType
    ALU = mybir.AluOpType

    nc = bacc.Bacc("TRN2", target_bir_lowering=False, debug=False,
                   num_devices=N_CORES)

    xT = nc.dram_tensor("xT", [D, S], f32r, kind="ExternalInput")
    w_qk = nc.dram_tensor("w_qk", [D, 384], f32r, kind="ExternalInput")
    b_qk = nc.dram_tensor("b_qk", [384, 1], f32, kind="ExternalInput")
    w_v = nc.dram_tensor("w_v", [D, 256], f32r, kind="ExternalInput")
    b_v = nc.dram_tensor("b_v", [1, 256], f32, kind="ExternalInput")
    w_o = nc.dram_tensor("w_o", [D, D], f32r, kind="ExternalInput")
    b_o = nc.dram_tensor("b_o", [D, 1], f32, kind="ExternalInput")
    gidx = nc.dram_tensor("gidx", [D, 1], u32, kind="ExternalInput")
    zin = nc.dram_tensor("zin", [P, P], f32r, kind="ExternalInput")
    out = nc.dram_tensor("out", [D, W], f32r, kind="ExternalOutput")

    cc_in = nc.dram_tensor("cc_in", [NQ, HPC * HD, W], f32r)
    cc_all = nc.dram_tensor("cc_all", [NQ * N_CORES * HPC * HD, W], f32r,
                            addr_space="Shared")

    with tile.TileContext(nc) as tc:
        with tc.tile_pool(name="const", bufs=1) as const, \
             tc.tile_pool(name="qkp", bufs=1) as qkp, \
             tc.tile_pool(name="vp", bufs=1) as vp, \
             tc.tile_pool(name="work", bufs=4) as work, \
             tc.tile_pool(name="expp", bufs=4) as expp, \
             tc.tile_pool(name="gat", bufs=1) as gat, \
             tc.tile_pool(name="outp", bufs=3) as outp:

            # ---- constant loads -------------------------------------------
            zeros_t = const.tile([P, P], f32r, tag="zeros")
            nc.sync.dma_start(out=zeros_t, in_=zin[:, :])
            wqk = []
            xt = []
            for k in range(KD):
                t = const.tile([P, 384], f32r, tag=f"wqk{k}")
                nc.sync.dma_start(out=t, in_=w_qk[k * P:(k + 1) * P, :])
                wqk.append(t)
            for k in range(KD):
                t = const.tile([P, S], f32r, tag=f"xt{k}", name=f"xt{k}")
                xt.append(t)
            for k in range(KD):
                nc.scalar.dma_start(out=xt[k][:, 0:1024],
                                    in_=xT[k * P:(k + 1) * P, 0:1024])
            for k in range(KD):
                nc.scalar.dma_start(out=xt[k][:, 1024:2048],
                                    in_=xT[k * P:(k + 1) * P, 1024:2048])
            wv = []
            for k in range(KD):
                t = const.tile([P, 256], f32r, tag=f"wv{k}")
                nc.sync.dma_start(out=t, in_=w_v[k * P:(k + 1) * P, :])
                wv.append(t)
            bqk = []
            for m in range(3):
                t = const.tile([P, 1], f32, tag=f"bqk{m}")
                nc.sync.dma_start(out=t, in_=b_qk[m * P:(m + 1) * P, :])
                bqk.append(t)
            bv = const.tile([P, 256], f32, tag="bv")
            bv_bcast = bass.AP(tensor=b_v[:, :].tensor, offset=0,
                               ap=[[0, P], [1, 256]])
            nc.gpsimd.dma_start(out=bv, in_=bv_bcast)
            wo = []
            bo = []
            gix = []
            for k in range(KD):
                t = const.tile([P, D], f32r, tag=f"wo{k}")
                nc.sync.dma_start(out=t, in_=w_o[k * P:(k + 1) * P, :])
                wo.append(t)
                t = const.tile([P, 1], f32, tag=f"bo{k}")
                nc.sync.dma_start(out=t, in_=b_o[k * P:(k + 1) * P, :])
                bo.append(t)
                t = const.tile([P, 1], u32, tag=f"gix{k}")
                nc.sync.dma_start(out=t, in_=gidx[k * P:(k + 1) * P, :])
                gix.append(t)

            # ---- attention -----------------------------------------------
            # Chunk = one [s_k 128, s_q 512] score block for one head.
            # Groups of 2 chunks share a 2-bank PSUM tile so one ACT exp
            # covers 1024 columns (amortizes the ~352-cycle ACT overhead).
            # Software-pipelined emission: mm_s(g+1) is emitted before
            # mm_c(g) so the PE never stalls behind the ACT.
            qkt = [qkp.tile([P, S], f32r, tag=f"qkt{m}", name=f"qkt{m}")
                   for m in range(3)]
            q2c = qkp.tile([64, S], f32r, tag="q2c")
            vsb = [vp.tile([P, 256], f32r, tag=f"v{s}", name=f"v{s}")
                   for s in range(SK)]

            def normalize(pc, nq, h):
                rec = work.tile([1, W], f32, tag="rec")
                nc.vector.reciprocal(rec[0:1, :], pc[64:65, :])
                rb = work.tile([64, W], f32, tag="rb")
                nc.gpsimd.partition_broadcast(rb, rec[:1, :])
                ctx = work.tile([64, W], f32r, tag="ctx")
                nc.vector.tensor_tensor(out=ctx, in0=pc[0:64, :], in1=rb,
                                        op=ALU.mult)
                nc.gpsimd.dma_start(
                    out=cc_in[nq, h * HD:(h + 1) * HD, :],
                    in_=ctx)
                norm_done.setdefault(nq, set()).add(h)
                if norm_done[nq] == {0, 1, 2}:
                    nc.gpsimd.collective_compute(
                        "AllGather", ALU.bypass,
                        ins=[cc_in[nq]],
                        outs=[cc_all[nq * 1536:(nq + 1) * 1536, :]],
                        replica_groups=[list(range(N_CORES))])

            # build group list: per nq, pair phase then solo phase
            groups = []
            for nq in range(NQ):
                for sk in range(SK):
                    groups.append({"nq": nq, "chunks": [(0, sk), (1, sk)],
                                   "last": False})
                for sk in range(0, SK, 2):
                    g = {"nq": nq, "chunks": [(2, sk), (2, sk + 1)],
                         "last": sk == SK - 2}
                    groups.append(g)

            pc_tiles = {}
            cnt = {}
            norm_done = {}
            ag_fired = set()

            def emit_mm_s(gi, grp):
                nq = grp["nq"]
                eps = psE.tile([P, 2 * W], f32, tag="ea" if gi % 2 == 0
                               else "eb", name=f"eps{gi}")
                for j, (h, sk) in enumerate(grp["chunks"]):
                    if h == 0:
                        lhsT = qkt[0][0:64, sk * P:(sk + 1) * P]
                        rhs = qkt[1][0:64, nq * W:(nq + 1) * W]
                    elif h == 1:
                        lhsT = qkt[0][64:128, sk * P:(sk + 1) * P]
                        rhs = qkt[1][64:128, nq * W:(nq + 1) * W]
                    else:
                        lhsT = qkt[2][0:64, sk * P:(sk + 1) * P]
                        rhs = q2c[:, nq * W:(nq + 1) * W]
                    nc.tensor.matmul(eps[:, j * W:(j + 1) * W], lhsT, rhs,
                                     start=True, stop=True)
                esb = expp.tile([P, 2 * W], f32r, tag="e", name=f"esb{gi}")
                nc.scalar.activation(esb, eps, AF.Exp)
                return esb

            def emit_mm_c(grp, esb):
                nq = grp["nq"]
                for j, (h, sk) in enumerate(grp["chunks"]):
                    key = (nq, h)
                    if key not in pc_tiles:
                        pc_tiles[key] = psC.tile([65, W], f32, tag="pc",
                                                 name=f"pc{nq}_{h}")
                        cnt[key] = 0
                    nc.tensor.matmul(
                        pc_tiles[key],
                        vsb[sk][:, h * 65:h * 65 + 65],
                        esb[:, j * W:(j + 1) * W],
                        start=(cnt[key] == 0), stop=(cnt[key] == SK - 1))
                    cnt[key] += 1
                    if cnt[key] == SK:
                        normalize(pc_tiles[key], nq, h)

            with tc.tile_pool(name="ps_proj", bufs=4, space="PSUM") as psP:

                def emit_qk_block(n):
                    for m in range(3):
                        ps = psP.tile([P, W], f32, tag="proj",
                                      name=f"psqk{n}_{m}")
                        first = n == 0 and m == 0
                        if first:
                            # zero-contribution warmup: keeps the PE busy
                            # while x DMAs land so HAM reaches 2.4GHz; the
                            # two regions cover [0:512] so has_written is
                            # clean for the real accumulation below
                            for d in range(24):
                                if d % 2 == 0:
                                    nc.tensor.matmul(
                                        ps[:, 0:384], zeros_t, wqk[0][:, :],
                                        start=(d == 0), stop=False,
                                        skip_group_check=True)
                                else:
                                    nc.tensor.matmul(
                                        ps[:, 384:512], zeros_t,
                                        wqk[1][:, 0:128],
                                        start=(d == 1), stop=False,
                                        skip_group_check=True)
                        for k in range(KD):
                            nc.tensor.matmul(
                                ps,
                                wqk[k][:, m * P:(m + 1) * P],
                                xt[k][:, n * W:(n + 1) * W],
                                start=(k == 0 and not first),
                                stop=(k == KD - 1),
                                skip_group_check=first)
                        nc.vector.tensor_scalar_add(
                            qkt[m][:, n * W:(n + 1) * W], ps, bqk[m])
                    nc.sync.dma_start(out=q2c[:, n * W:(n + 1) * W],
                                      in_=qkt[2][64:128, n * W:(n + 1) * W])

                def emit_v_block(n):
                    for s_ in range(4 * n, 4 * n + 4):
                        ps = psP.tile([P, W], f32, tag="proj",
                                      name=f"psv{s_}")
                        for k in range(KD):
                            nc.tensor.matmul(
                                ps[:, 0:256],
                                xt[k][:, s_ * P:(s_ + 1) * P],
                                wv[k],
                                start=(k == 0), stop=(k == KD - 1))
                        nc.vector.tensor_tensor(out=vsb[s_], in0=ps[:, 0:256],
                                                in1=bv, op=ALU.add)

                for n in range(NQ):
                    emit_qk_block(n)
                    emit_v_block(n)

            with tc.tile_pool(name="ps_e", bufs=1, space="PSUM") as psE, \
                 tc.tile_pool(name="ps_c", bufs=3, space="PSUM") as psC:
                prev = None
                for gi, grp in enumerate(groups):
                    esb = emit_mm_s(gi, grp)
                    if prev is not None:
                        emit_mm_c(prev[0], prev[1])
                    prev = (grp, esb)
                emit_mm_c(prev[0], prev[1])

            # ---- gather + output projection ------------------------------
            ctxg = []
            for k in range(KD):
                t = gat.tile([P, W], f32r, tag=f"ctxg{k}", name=f"ctxg{k}")
                nc.gpsimd.indirect_dma_start(
                    out=t,
                    out_offset=None,
                    in_=cc_all[:, :],
                    in_offset=bass.IndirectOffsetOnAxis(ap=gix[k][:, :1],
                                                        axis=0),
                )
                ctxg.append(t)
            with tc.tile_pool(name="ps_y", bufs=2, space="PSUM") as py:
                for m in range(KD):
                    ps = py.tile([P, W], f32)
                    if m == 0:
                        # warm the PE during the AllGather wait so the
                        # output projection runs at full clock
                        for d in range(16):
                            nc.tensor.matmul(
                                ps, zeros_t, wo[0][:, 0:W],
                                start=(d == 0), stop=False,
                                skip_group_check=True)
                    for k in range(KD):
                        nc.tensor.matmul(
                            ps,
                            wo[k][:, m * P:(m + 1) * P],
                            ctxg[k],
                            start=(k == 0 and m != 0),
                            stop=(k == KD - 1),
                            skip_group_check=(m == 0))
                    yt = outp.tile([P, W], f32r, tag="yt")
                    nc.vector.tensor_scalar_add(yt, ps, bo[m])
                    nc.gpsimd.dma_start(out=out[m * P:(m + 1) * P, :], in_=yt)

    nc.compile()
    return nc


def _get_nc():
    if "nc" not in _CACHE:
        _install_profile_shim()
        _CACHE["nc"] = _build()
    return _CACHE["nc"]


def _make_in_maps(x, Wq, bq, Wk, bk, Wv, bv, Wo, bo):
    scale = np.float32(1.0 / np.sqrt(HD))
    f = np.float32
    x, Wq, bq, Wk, bk, Wv, bv, Wo, bo = [
        np.asarray(a, dtype=f) for a in (x, Wq, bq, Wk, bk, Wv, bv, Wo, bo)]

    in_maps = []
    for c in range(N_CORES):
        b = c // 4
        hs = (c % 4) * HPC
        q = c % 4
        hh = [hs, hs + 1, hs + 2]

        def wc(Wm, h):
            return Wm[:, h * HD:(h + 1) * HD]

        def bc(bm, h):
            return bm[h * HD:(h + 1) * HD]

        xTb = np.ascontiguousarray(x[b].T)
        w_qk = np.concatenate(
            [wc(Wk, hh[0]), wc(Wk, hh[1]),
             wc(Wq, hh[0]) * scale, wc(Wq, hh[1]) * scale,
             wc(Wk, hh[2]), wc(Wq, hh[2]) * scale], axis=1)
        b_qk = np.concatenate(
            [bc(bk, hh[0]), bc(bk, hh[1]),
             bc(bq, hh[0]) * scale, bc(bq, hh[1]) * scale,
             bc(bk, hh[2]), bc(bq, hh[2]) * scale])[:, None]
        w_v = np.zeros((D, 256), dtype=f)
        b_v = np.zeros((1, 256), dtype=f)
        for i, h in enumerate(hh):
            w_v[:, i * 65:i * 65 + HD] = wc(Wv, h)
            b_v[0, i * 65:i * 65 + HD] = bc(bv, h)
            b_v[0, i * 65 + HD] = 1.0
        i_feat = np.arange(D, dtype=np.uint32)
        g = q * 1536 + (4 * b + i_feat // 192) * 192 + (i_feat % 192)
        in_maps.append({
            "xT": np.ascontiguousarray(xTb),
            "w_qk": np.ascontiguousarray(w_qk),
            "b_qk": np.ascontiguousarray(b_qk),
            "w_v": w_v,
            "b_v": b_v,
            "w_o": np.ascontiguousarray(Wo),
            "b_o": np.ascontiguousarray(bo[:, None]),
            "gidx": g.astype(np.uint32)[:, None],
            "zin": np.zeros((P, P), dtype=f),
        })
    return in_maps


def kernel(x, Wq, bq, Wk, bk, Wv, bv, Wo, bo, _trace=False):
    from concourse.bass_utils import run_bass_kernel_spmd

    nc = _get_nc()
    in_maps = _make_in_maps(x, Wq, bq, Wk, bk, Wv, bv, Wo, bo)
    res = run_bass_kernel_spmd(nc, in_maps, list(range(N_CORES)),
                               trace=_trace)
    _CACHE["last_results"] = res
    y = np.empty((B, S, D), dtype=np.float32)
    for c in range(N_CORES):
        b = c // 4
        q = c % 4
        y[b, q * W:(q + 1) * W, :] = res.results[c]["out"].T
    return y
